# revision 1
# baseline (speedup 1.0000x reference)
"""Bass/Trainium2 kernel v2 for nn_BlockForNormalWindow (windowed-attention
transformer block), data-parallel over batch across 8 NeuronCores.

v2: bf16 datapath, SBUF-resident q/rel and v operands, diagonal-only
windowed scores (no cross-window masking), merged DMAs."""
import sys
sys.path.insert(0, '/opt/trn_rl_repo')

import numpy as np
import ml_dtypes
import concourse.bass as bass
import concourse.mybir as mybir
import concourse.tile as tile
from concourse import bacc
from concourse.bass_utils import run_bass_kernel_spmd
from concourse.masks import make_identity

F32 = mybir.dt.float32
BF = mybir.dt.bfloat16
AF = mybir.ActivationFunctionType
ALU = mybir.AluOpType

B, H, W = 8, 64, 64
DIM, NH, WS = 384, 6, 14
HD = DIM // NH
MLP = 4 * DIM
EPS = 1e-5
SCALE = HD ** -0.5
HP = 70
NWIN = 25
NTOK = NWIN * WS * WS        # 4900
NVAL = H * W                 # 4096
VS = 66                      # per-head stride in v layout (64 vals + ones + pad)
KR = 128                     # rows in k/q operand: q/k 0:64, relh 64:78, relw 96:110, rest zero

GROUPS = [(g * 392, 392) for g in range(12)] + [(4704, 196)]


def _win_origin(w):
    return (w // 5) * 14 * HP + (w % 5) * 14


def _ap(t, offset_elems, dims, p=None):
    a = t[:, 0:1] if p is None else t[p[0]:p[1], 0:1]
    return bass.AP(tensor=a.tensor, offset=a.offset + offset_elems,
                   ap=[a.ap[0]] + dims)


def _dram_ap(t, offset_elems, dims):
    a = t.ap()
    return bass.AP(tensor=a.tensor, offset=offset_elems, ap=dims)


def build_bass():
    nc = bacc.Bacc("TRN2", target_bir_lowering=False, debug=False)

    x_in = nc.dram_tensor("x", [NVAL, DIM], F32, kind="ExternalInput")
    wqk_in = nc.dram_tensor("wqk", [DIM, 2 * DIM], BF, kind="ExternalInput")
    bqk_in = nc.dram_tensor("bqk", [2 * DIM], F32, kind="ExternalInput")
    wv_in = nc.dram_tensor("wv", [DIM, DIM], BF, kind="ExternalInput")
    rel_in = nc.dram_tensor("rel", [HD, 2 * 196], BF, kind="ExternalInput")
    kpat_in = nc.dram_tensor("kpat", [64, 392], BF, kind="ExternalInput")
    wp_in = nc.dram_tensor("wp", [DIM, DIM], BF, kind="ExternalInput")
    bp_in = nc.dram_tensor("bp", [DIM], F32, kind="ExternalInput")
    w1_in = nc.dram_tensor("w1", [DIM, MLP], BF, kind="ExternalInput")
    b1_in = nc.dram_tensor("b1", [MLP], F32, kind="ExternalInput")
    w2_in = nc.dram_tensor("w2", [MLP, DIM], BF, kind="ExternalInput")
    b2_in = nc.dram_tensor("b2", [DIM], F32, kind="ExternalInput")
    out_d = nc.dram_tensor("out", [NVAL, DIM], F32, kind="ExternalOutput")

    # k operand in DRAM: [KR, NH, NTOK]; rows 64:92 hold the rel-select
    # pattern (written once from kpat_in), rows 0:64 written by phase B.
    kT_d = nc.dram_tensor("kT_d", [KR, NH * NTOK], BF)
    y_d = nc.dram_tensor("y_d", [HP * HP, DIM], BF)

    with tile.TileContext(nc) as tc:
      with tc.tile_pool(name="singles", bufs=1) as singles:
        ident_f = singles.tile([128, 128], F32)
        make_identity(nc, ident_f[:])
        identB = singles.tile([128, 128], BF)
        nc.vector.tensor_copy(out=identB[:], in_=ident_f[:])

        eps_t = singles.tile([128, 1], F32)
        nc.vector.memset(eps_t[:], EPS)
        ones1_t = singles.tile([1, 128], BF)
        nc.vector.memset(ones1_t[:], 1.0)

        with tc.tile_pool(name="attops", bufs=1) as attops:
          with tc.tile_pool(name="pHT", bufs=1) as pHT:
            hT = [pHT.tile([128, NTOK], BF, name=f"hT{c}") for c in range(3)]
            for c in range(3):
                nc.vector.memset(hT[c][:, 64 * HP:HP * HP], 0.0)
                nc.vector.memset(_ap(hT[c], 64, [[HP, 64], [1, 6]]), 0.0)

            # ===== Phase A: LN1 + transpose into hT =====
            with tc.tile_pool(name="pA", bufs=4) as pA, \
                 tc.tile_pool(name="pA_ps", bufs=4, space="PSUM") as pA_ps:
                for ch in range(8):
                    xc = pA.tile([128, 4, DIM], F32, tag="xc")
                    nc.sync.dma_start(
                        out=xc[:],
                        in_=_dram_ap(x_in, 512 * ch * DIM,
                                     [[DIM, 128], [128 * DIM, 4], [1, DIM]]))
                    mvall = pA.tile([128, 4, 2], F32, tag="mva")
                    for tt in range(4):
                        stats = pA.tile([128, 6], F32, tag="st")
                        nc.vector.bn_stats(out=stats[:], in_=xc[:, tt, :])
                        nc.vector.bn_aggr(out=mvall[:, tt, :], in_=stats[:])
                    rstd = pA.tile([128, 4], F32, tag="rstd")
                    nc.scalar.activation(out=rstd[:], in_=_ap(mvall, 1, [[2, 4]]),
                                         func=AF.Sqrt, bias=eps_t[:], scale=1.0)
                    nc.vector.reciprocal(out=rstd[:], in_=rstd[:])
                    for tt in range(4):
                        t = 4 * ch + tt
                        nmr = pA.tile([128, 1], F32, tag="nmr")
                        nc.vector.scalar_tensor_tensor(out=nmr[:], in0=mvall[:, tt, 0:1],
                                                       scalar=-1.0, in1=rstd[:, tt:tt + 1],
                                                       op0=ALU.mult, op1=ALU.mult)
                        hn = pA.tile([128, DIM], BF, tag="hn")
                        nc.gpsimd.tensor_scalar(out=hn[:], in0=xc[:, tt, :],
                                                scalar1=nmr[:],
                                                scalar2=rstd[:, tt:tt + 1],
                                                op0=ALU.add, op1=ALU.mult)
                        for c in range(3):
                            pt = pA_ps.tile([128, 128], BF, tag="tr")
                            nc.tensor.transpose(pt[:], hn[:, c * 128:(c + 1) * 128],
                                                identB[:])
                            dst = _ap(hT[c], 2 * t * HP, [[HP, 2], [1, 64]])
                            if (t * 3 + c) % 3 == 0:
                                nc.vector.tensor_copy(out=dst, in_=pt[:])
                            else:
                                nc.scalar.copy(out=dst, in_=pt[:])

            bqk_t = singles.tile([128, 6], F32)
            nc.sync.dma_start(out=bqk_t[:], in_=bqk_in.ap().rearrange("(m p) -> p m", p=128))
            bp_t = singles.tile([128, 3], F32)
            nc.sync.dma_start(out=bp_t[:], in_=bp_in.ap().rearrange("(m p) -> p m", p=128))
            b1_t = singles.tile([128, 12], F32)
            nc.sync.dma_start(out=b1_t[:], in_=b1_in.ap().rearrange("(m p) -> p m", p=128))
            b2_t = singles.tile([128, 3], F32)
            nc.sync.dma_start(out=b2_t[:], in_=b2_in.ap().rearrange("(m p) -> p m", p=128))

            bpf = singles.tile([1, DIM], F32)
            nc.sync.dma_start(out=bpf[:], in_=bp_in.ap())
            bprow = singles.tile([1, DIM], BF)
            nc.vector.tensor_copy(out=bprow[:], in_=bpf[:])
            b2f = singles.tile([1, DIM], F32)
            nc.sync.dma_start(out=b2f[:], in_=b2_in.ap())
            b2row = singles.tile([1, DIM], BF)
            nc.vector.tensor_copy(out=b2row[:], in_=b2f[:])

            wqk_t = singles.tile([128, 3, 2 * DIM], BF)
            nc.sync.dma_start(out=wqk_t[:], in_=wqk_in.ap().rearrange("(kc p) n -> p kc n", p=128))
            wv_t = singles.tile([128, 3, DIM], BF)
            nc.sync.dma_start(out=wv_t[:], in_=wv_in.ap().rearrange("(kc p) n -> p kc n", p=128))
            relm_t = singles.tile([HD, 2 * 196], BF)
            nc.sync.dma_start(out=relm_t[:], in_=rel_in.ap())
            wp_t = singles.tile([128, 3, DIM], BF)
            nc.sync.dma_start(out=wp_t[:], in_=wp_in.ap().rearrange("(kc p) n -> p kc n", p=128))
            w1_t = singles.tile([128, 3, MLP], BF)
            nc.sync.dma_start(out=w1_t[:], in_=w1_in.ap().rearrange("(kc p) n -> p kc n", p=128))
            w2_t = singles.tile([128, 12, DIM], BF)
            nc.sync.dma_start(out=w2_t[:], in_=w2_in.ap().rearrange("(kc p) n -> p kc n", p=128))

            # persistent kTa ring: pattern rows written once, k rows per group
            kta2 = [attops.tile([KR, NH, 392], BF, name=f"kta{i}") for i in range(2)]
            for i in range(2):
                nc.sync.dma_start(
                    out=kta2[i][64:128, :, :],
                    in_=bass.AP(tensor=kpat_in.ap().tensor, offset=0,
                                ap=[[392, 64], [0, 6], [1, 392]]))


            # persistent attention operands (allocated after phase A pools
            # release, so A's x staging reuses this space)
            qb = [attops.tile([KR, NTOK], BF, name=f"qb{h}") for h in range(NH)]
            for h in range(NH):
                nc.scalar.memzero(qb[h][64:128, :])
            vt = [attops.tile([98, NH * VS], BF, name=f"vt{s}") for s in range(50)]
            for s in range(50):
                e = nc.vector if s % 2 == 0 else nc.gpsimd
                e.memset(_ap(vt[s], 64, [[VS, 6], [1, 1]]), 1.0)
                e.memset(_ap(vt[s], 65, [[VS, 6], [1, 1]]), 0.0)

            # ===== Phase B: qk (+rel per head-pair) and v =====
            with tc.tile_pool(name="pB", bufs=3) as pB, \
                 tc.tile_pool(name="pB_ps", bufs=3, space="PSUM") as pB_ps, \
                 tc.tile_pool(name="pC_ps", bufs=2, space="PSUM") as pC_ps, \
                 tc.tile_pool(name="pBv_ps", bufs=2, space="PSUM") as pBv_ps:
                def emit_v(s_):
                    w = (98 * s_) // 196
                    r0 = ((98 * s_) % 196) // 14
                    ov = _win_origin(w) + r0 * HP
                    ps = pBv_ps.tile([98, DIM], F32, tag="v", name="vps")
                    hstage = pB.tile([128, 3, 98], BF, tag="hstage", name="hstage")
                    for kc in range(3):
                        if kc < 2:
                            nc.vector.tensor_copy(out=hstage[:, kc, :],
                                                  in_=_ap(hT[kc], ov, [[HP, 7], [1, 14]]))
                        else:
                            nc.scalar.copy(out=hstage[:, kc, :],
                                           in_=_ap(hT[kc], ov, [[HP, 7], [1, 14]]))
                    for kc in range(3):
                        nc.tensor.matmul(ps[:], hstage[:, kc, :], wv_t[:, kc, :],
                                         start=(kc == 0), stop=(kc == 2))
                    dst = _ap(vt[s_], 0, [[VS, 6], [1, 64]])
                    if s_ % 2 == 0:
                        nc.scalar.copy(out=dst, in_=ps[:])
                    else:
                        nc.vector.tensor_copy(out=dst, in_=ps[:])
                vnext = [0]
                kst_cur = [None]
                for m in range(6):
                    for gi, (p0, plen) in enumerate(GROUPS):
                        w0 = 2 * gi
                        o0 = _win_origin(w0)
                        if plen == 392:
                            dims = [[_win_origin(w0 + 1) - o0, 2], [HP, 14], [1, 14]]
                        else:
                            dims = [[HP, 14], [1, 14]]
                        ps = pB_ps.tile([128, 392], F32, tag="qk")
                        for kc in range(3):
                            nc.tensor.matmul(ps[:, 0:plen],
                                             wqk_t[:, kc, m * 128:(m + 1) * 128],
                                             _ap(hT[kc], o0, dims),
                                             start=(kc == 0), stop=(kc == 2))
                        if m < 3:
                            for half in range(2):
                                h = 2 * m + half
                                dst = qb[h][0:64, p0:p0 + plen]
                                src = ps[64 * half:64 * half + 64, 0:plen]
                                bia = bqk_t[64 * half:64 * half + 64, m:m + 1]
                                if (gi + half) % 2 == 0:
                                    nc.scalar.activation(out=dst, in_=src,
                                                         func=AF.Identity, bias=bia,
                                                         scale=1.0)
                                else:
                                    nc.vector.tensor_scalar(out=dst, in0=src,
                                                            scalar1=bia, scalar2=None,
                                                            op0=ALU.add)
                        else:
                            mm = m - 3
                            if gi % 2 == 0:
                                kst = pB.tile([128, 2, 392], BF, tag="kst", bufs=4,
                                              name="kst")
                                kst_cur[0] = kst
                            else:
                                kst = kst_cur[0]
                            half = gi % 2
                            if gi % 2 == 0:
                                nc.scalar.activation(out=kst[:, half, 0:plen],
                                                     in_=ps[:, 0:plen],
                                                     func=AF.Identity,
                                                     bias=bqk_t[:, m:m + 1], scale=1.0)
                            else:
                                nc.vector.tensor_scalar(out=kst[:, half, 0:plen],
                                                        in0=ps[:, 0:plen],
                                                        scalar1=bqk_t[:, m:m + 1],
                                                        scalar2=None, op0=ALU.add)
                            if gi % 2 == 1:
                                nc.gpsimd.dma_start(
                                    out=_dram_ap(kT_d, 2 * mm * NTOK + p0 - 392,
                                                 [[NTOK, 2], [NH * NTOK, 64], [1, 784]]),
                                    in_=kst[:])
                            elif gi == 12:
                                nc.gpsimd.dma_start(
                                    out=_dram_ap(kT_d, 2 * mm * NTOK + p0,
                                                 [[NTOK, 2], [NH * NTOK, 64], [1, plen]]),
                                    in_=kst[:, 0, 0:plen])
                        if vnext[0] < 50 and (m, gi) != (0, 0):
                            emit_v(vnext[0])
                            vnext[0] += 1
                    if m < 3:
                        # rel rows for heads 2m, 2m+1 (q complete for them now)
                        for half in range(2):
                            h = 2 * m + half
                            for r in range(14):
                                ps = pC_ps.tile([14, 350], F32, tag="rel")
                                nc.tensor.matmul(
                                    ps[:], relm_t[:, r * 14:(r + 1) * 14],
                                    _ap(qb[h], r * 14, [[196, 25], [1, 14]], p=(0, 64)),
                                    start=True, stop=True)
                                dst = _ap(qb[h], r * 14, [[196, 25], [1, 14]], p=(64, 78))
                                if r % 2 == 0:
                                    nc.scalar.copy(out=dst, in_=ps[:])
                                else:
                                    nc.vector.tensor_copy(out=dst, in_=ps[:])
                            for cc in range(14):
                                ps = pC_ps.tile([14, 350], F32, tag="rel")
                                nc.tensor.matmul(
                                    ps[:], relm_t[:, 196 + cc * 14:196 + (cc + 1) * 14],
                                    _ap(qb[h], cc, [[196, 25], [14, 14]], p=(0, 64)),
                                    start=True, stop=True)
                                dst = _ap(qb[h], cc, [[196, 25], [14, 14]], p=(96, 110))
                                if cc % 2 == 0:
                                    nc.vector.tensor_copy(out=dst, in_=ps[:])
                                else:
                                    nc.scalar.copy(out=dst, in_=ps[:])

          # ===== Phase D: attention + proj (hT freed) =====
          with tc.tile_pool(name="pD", bufs=4) as pD, \
               tc.tile_pool(name="pDet", bufs=8) as pDet, \
               tc.tile_pool(name="pDa", bufs=3) as pDa, \
               tc.tile_pool(name="pDs_ps", bufs=3, space="PSUM") as pDs_ps, \
               tc.tile_pool(name="pDo_ps", bufs=3, space="PSUM") as pDo_ps, \
               tc.tile_pool(name="pDp_ps", bufs=2, space="PSUM") as pDp_ps:
            for gi, (p0, plen) in enumerate(GROUPS):
                nwin = plen // 196
                kTa = kta2[gi % 2]
                nc.sync.dma_start(
                    out=kTa[0:64, :, 0:plen],
                    in_=_dram_ap(kT_d, p0,
                                 [[NH * NTOK, 64], [NTOK, NH], [1, plen]]))
                attnT = [pDa.tile([128, 392], BF, tag=f"attnT{c}", name=f"attnT{c}")
                         for c in range(3)]
                for h in range(NH):
                    ets = []
                    for i in range(nwin):
                        st = pDs_ps.tile([98, 2, 196], F32, tag="st")
                        for j in range(2):
                            nc.tensor.matmul(
                                st[:, j, :],
                                kTa[:, h, 196 * i + 98 * j:196 * i + 98 * j + 98],
                                qb[h][:, p0 + 196 * i:p0 + 196 * i + 196],
                                start=True, stop=True)
                        et = pDet.tile([98, 2, 196], BF, tag="et")
                        nc.scalar.activation(out=et[:], in_=st[:], func=AF.Exp,
                                             bias=0.0, scale=1.0)
                        ets.append(et)
                    oT = pDo_ps.tile([VS, 2, 196], F32, tag="oT")
                    for i in range(nwin):
                        for j in range(2):
                            s = 4 * gi + 2 * i + j
                            nc.tensor.matmul(oT[:, i, :],
                                             vt[s][:, h * VS:(h + 1) * VS],
                                             ets[i][:, j, :],
                                             start=(j == 0), stop=(j == 1))
                    rz = pD.tile([1, 392], F32, tag="rz")
                    nc.vector.reciprocal(out=rz[:, 0:plen], in_=oT[64:65, 0:nwin, :])
                    rzb = pD.tile([64, 392], F32, tag="rzb")
                    nc.gpsimd.partition_broadcast(rzb[:, 0:plen], rz[:, 0:plen])
                    dst = attnT[h // 2][(h % 2) * 64:(h % 2) * 64 + 64, 0:plen]
                    nc.vector.tensor_tensor(out=dst, in0=oT[0:64, 0:nwin, :],
                                            in1=rzb[:, 0:plen], op=ALU.mult)
                for i in range(nwin):
                    w = 2 * gi + i
                    for jj in range(2):
                        pj = pDp_ps.tile([98, DIM], F32, tag="pj")
                        sl = 196 * i + 98 * jj
                        for kc in range(3):
                            nc.tensor.matmul(pj[:], attnT[kc][:, sl:sl + 98],
                                             wp_t[:, kc, :],
                                             start=(kc == 0), stop=False)
                        nc.tensor.matmul(pj[:], ones1_t[:, 0:98], bprow[:],
                                         start=False, stop=True)
                        ysb = pD.tile([98, DIM], BF, tag="ysb")
                        if jj == 0:
                            nc.vector.tensor_copy(out=ysb[:], in_=pj[:])
                        else:
                            nc.scalar.copy(out=ysb[:], in_=pj[:])
                        e = nc.gpsimd if jj == 0 else nc.scalar
                        e.dma_start(
                            out=_dram_ap(y_d, (_win_origin(w) + 7 * jj * HP) * DIM,
                                         [[HP * DIM, 7], [DIM, 14], [1, DIM]]),
                            in_=ysb[:])
        # ===== Phase E: residual + LN2 + MLP (attention operands freed) =====
        with tc.tile_pool(name="pE", bufs=3) as pE, \
             tc.tile_pool(name="pEz", bufs=3) as pEz, \
             tc.tile_pool(name="pEh", bufs=3) as pEh, \
             tc.tile_pool(name="pEg", bufs=3) as pEg, \
             tc.tile_pool(name="pE_ps", bufs=2, space="PSUM") as pE_ps, \
             tc.tile_pool(name="pE2_ps", bufs=3, space="PSUM") as pE2_ps, \
             tc.tile_pool(name="pE3_ps", bufs=3, space="PSUM") as pE3_ps:
            def e_group(g):
                xc = pE.tile([128, 4, DIM], F32, tag="xe", name="xc")
                nc.sync.dma_start(
                    out=xc[:],
                    in_=_dram_ap(x_in, 512 * g * DIM,
                                 [[DIM, 128], [128 * DIM, 4], [1, DIM]]))
                yc = pE.tile([128, 4, DIM], BF, tag="ye", name="yc")
                for tt in range(4):
                    nc.sync.dma_start(
                        out=yc[:, tt, :],
                        in_=_dram_ap(y_d, (8 * g + 2 * tt) * HP * DIM,
                                     [[HP * DIM, 2], [DIM, 64], [1, DIM]]))
                zts = []
                mvall = pE.tile([128, 4, 2], F32, tag="mva", name="mvall")
                for tt in range(4):
                    zt = pEz.tile([128, DIM], BF, tag=f"ze{tt}", name="zt")
                    zts.append(zt)
                    nc.gpsimd.tensor_tensor(out=zt[:], in0=xc[:, tt, :],
                                            in1=yc[:, tt, :], op=ALU.add)
                    stats = pE.tile([128, 6], F32, tag="st_e", name="stats")
                    nc.vector.bn_stats(out=stats[:], in_=zt[:])
                    nc.vector.bn_aggr(out=mvall[:, tt, :], in_=stats[:])
                rstd = pE.tile([128, 4], F32, tag="rstd_e", name="rstd")
                nc.scalar.activation(out=rstd[:], in_=_ap(mvall, 1, [[2, 4]]),
                                     func=AF.Sqrt, bias=eps_t[:], scale=1.0)
                nc.vector.reciprocal(out=rstd[:], in_=rstd[:])
                h2T = [pEh.tile([128, 512], BF, tag=f"h2T{c}", name=f"h2T{c}")
                       for c in range(3)]
                for tt in range(4):
                    nmr = pE.tile([128, 1], F32, tag="nmr_e", name="nmr")
                    nc.vector.scalar_tensor_tensor(out=nmr[:], in0=mvall[:, tt, 0:1],
                                                   scalar=-1.0, in1=rstd[:, tt:tt + 1],
                                                   op0=ALU.mult, op1=ALU.mult)
                    hn = pEz.tile([128, DIM], BF, tag=f"hn{tt}", name="hn")
                    nc.gpsimd.tensor_scalar(out=hn[:], in0=zts[tt][:], scalar1=nmr[:],
                                            scalar2=rstd[:, tt:tt + 1], op0=ALU.add,
                                            op1=ALU.mult)
                    for c in range(3):
                        pt = pE_ps.tile([128, 128], BF, tag="htr", name="pt")
                        nc.tensor.transpose(pt[:], hn[:, c * 128:(c + 1) * 128],
                                            identB[:])
                        dst = h2T[c][:, tt * 128:(tt + 1) * 128]
                        nc.vector.tensor_copy(out=dst, in_=pt[:])
                gt = []
                for m in range(12):
                    ps = pE2_ps.tile([128, 512], F32, tag="fc1", name="ps1")
                    for kc in range(3):
                        nc.tensor.matmul(ps[:], w1_t[:, kc, m * 128:(m + 1) * 128],
                                         h2T[kc][:], start=(kc == 0), stop=(kc == 2))
                    gm = pEg.tile([128, 512], BF, tag=f"g{m}", name=f"g{m}")
                    nc.scalar.activation(out=gm[:], in_=ps[:], func=AF.Gelu,
                                         bias=b1_t[:, m:m + 1], scale=1.0)
                    gt.append(gm)
                ot = pE.tile([128, 4, DIM], F32, tag="oe", name="ot")
                for tt in range(4):
                    ps = pE3_ps.tile([128, DIM], F32, tag="fc2", name="ps2")
                    for kc in range(12):
                        nc.tensor.matmul(ps[:], gt[kc][:, tt * 128:(tt + 1) * 128],
                                         w2_t[:, kc, :],
                                         start=(kc == 0), stop=False)
                    nc.tensor.matmul(ps[:], ones1_t[:], b2row[:],
                                     start=False, stop=True)
                    nc.vector.tensor_tensor(out=ot[:, tt, :], in0=zts[tt][:],
                                            in1=ps[:], op=ALU.add)
                nc.sync.dma_start(
                    out=_dram_ap(out_d, 512 * g * DIM,
                                 [[DIM, 128], [128 * DIM, 4], [1, DIM]]),
                    in_=ot[:])

            for g in range(8):
                e_group(g)

    nc.compile()
    return nc


_NC = None


def _get_nc():
    global _NC
    if _NC is None:
        _NC = build_bass()
    return _NC


def _host_prep(inputs):
    f = np.float32
    bf = ml_dtypes.bfloat16
    ln1_w = np.asarray(inputs["ln1_w"], f); ln1_b = np.asarray(inputs["ln1_b"], f)
    qkv_w = np.asarray(inputs["qkv_w"], f); qkv_b = np.asarray(inputs["qkv_b"], f)
    proj_w = np.asarray(inputs["proj_w"], f); proj_b = np.asarray(inputs["proj_b"], f)
    ln2_w = np.asarray(inputs["ln2_w"], f); ln2_b = np.asarray(inputs["ln2_b"], f)
    fc1_w = np.asarray(inputs["fc1_w"], f); fc1_b = np.asarray(inputs["fc1_b"], f)
    fc2_w = np.asarray(inputs["fc2_w"], f); fc2_b = np.asarray(inputs["fc2_b"], f)
    rel_h = np.asarray(inputs["rel_pos_h"], f); rel_w = np.asarray(inputs["rel_pos_w"], f)

    wqk = (ln1_w[:, None] * qkv_w[:, :768]).copy()
    bqk = (ln1_b @ qkv_w[:, :768] + qkv_b[:768]).copy()
    wqk[:, :384] *= SCALE
    bqk[:384] *= SCALE
    wv = (ln1_w[:, None] * qkv_w[:, 768:]).copy()
    bv = ln1_b @ qkv_w[:, 768:] + qkv_b[768:]

    coords = np.arange(WS)[:, None] - np.arange(WS)[None, :] + (WS - 1)
    Rh = rel_h[coords]
    Rw = rel_w[coords]
    rel = np.zeros((HD, 2 * 196), f)
    for r in range(14):
        rel[:, r * 14:(r + 1) * 14] = Rh[r].T / SCALE
    for c in range(14):
        rel[:, 196 + c * 14:196 + (c + 1) * 14] = Rw[c].T / SCALE

    kpat = np.zeros((64, 392), f)
    for j in range(14):
        for a in range(2):
            kpat[j, 196 * a + 14 * j:196 * a + 14 * j + 14] = 1.0
            kpat[32 + j, 196 * a + j::14][:14] = 1.0

    return {
        "wqk": np.ascontiguousarray(wqk).astype(bf),
        "bqk": np.ascontiguousarray(bqk, f),
        "wv": np.ascontiguousarray(wv).astype(bf),
        "rel": rel.astype(bf),
        "kpat": kpat.astype(bf),
        "wp": np.ascontiguousarray(proj_w).astype(bf),
        "bp": np.ascontiguousarray(proj_b + bv @ proj_w, f),
        "w1": np.ascontiguousarray(ln2_w[:, None] * fc1_w).astype(bf),
        "b1": np.ascontiguousarray(ln2_b @ fc1_w + fc1_b, f),
        "w2": np.ascontiguousarray(fc2_w).astype(bf),
        "b2": np.ascontiguousarray(fc2_b, f),
    }


def kernel(**inputs):
    nc = _get_nc()
    shared = _host_prep(inputs)
    x = np.asarray(inputs["x"], np.float32).reshape(B, NVAL, DIM)
    in_maps = [dict(shared, x=np.ascontiguousarray(x[c])) for c in range(B)]
    res = run_bass_kernel_spmd(nc, in_maps, list(range(B)))
    out = np.stack([res.results[c]["out"] for c in range(B)])
    return out.reshape(B, H, W, DIM)


if __name__ == "__main__":
    build_bass()
    print("build ok")



# revision 11
# speedup vs baseline: 1.1022x; 1.1022x over previous
"""Bass/Trainium2 kernel v3 for nn_BlockForNormalWindow (windowed-attention
transformer block), data-parallel over batch across 8 NeuronCores.

v3 over v2: fp8e4 DoubleRow matmuls for qkv/v/proj/fc1/fc2 (weights x64,
biases folded via ones-row in a 4th K-chunk), window-major fp8 hT
(contiguous group slices, no hstage), bf16 score path with K=92 layout
(no garbage rows), fp8 ets/v with DoubleRow AV, PE-broadcast of 1/z,
Quake rsqrt on DVE for LN2 (no act-table thrash), 3-way engine rotation
for PSUM->SBUF copies, split E1/E2 MLP phase."""
import sys
sys.path.insert(0, '/opt/trn_rl_repo')

import numpy as np
import ml_dtypes
import concourse.bass as bass
import concourse.mybir as mybir
import concourse.tile as tile
from concourse import bacc
from concourse.bass_utils import run_bass_kernel_spmd
from concourse.masks import make_identity

F32 = mybir.dt.float32
I32 = mybir.dt.int32
BF = mybir.dt.bfloat16
F8 = mybir.dt.float8e4
AF = mybir.ActivationFunctionType
ALU = mybir.AluOpType
DR = mybir.MatmulPerfMode.DoubleRow

B, H, W = 8, 64, 64
DIM, NH, WS = 384, 6, 14
HD = DIM // NH
MLP = 4 * DIM
EPS = 1e-5
SCALE = HD ** -0.5
HP = 70
NWIN = 25
NTOK = NWIN * WS * WS        # 4900
NVAL = H * W                 # 4096
VS = 65                      # per-head stride in v layout (64 vals + ones col)
KR = 92                      # rows in k/q operand: q/k 0:64, relh 64:78, relw 78:92
FS = 64.0                    # fp8 weight pre-scale
FSI = 1.0 / FS
QMAGIC = 1.3211836172961055e+19   # 0x5f3759df as float32

GROUPS = [(g * 392, 392) for g in range(12)] + [(4704, 196)]
# E1 group g (image rows 8g:8g+8) ready after this D group index
E1_AFTER = {2: [0], 4: [1, 2], 7: [3, 4], 9: [5, 6], 12: [7]}


def _ap(t, offset_elems, dims, p=None):
    a = t[:, 0:1] if p is None else t[p[0]:p[1], 0:1]
    return bass.AP(tensor=a.tensor, offset=a.offset + offset_elems,
                   ap=[a.ap[0]] + dims)


def _dram_ap(t, offset_elems, dims):
    a = t.ap()
    return bass.AP(tensor=a.tensor, offset=offset_elems, ap=dims)


def build_bass():
    nc = bacc.Bacc("TRN2", target_bir_lowering=False, debug=False)

    x_in = nc.dram_tensor("x", [NVAL, DIM], F32, kind="ExternalInput")
    wqk_in = nc.dram_tensor("wqk", [128, 4 * 2 * DIM], F8, kind="ExternalInput")
    wv_in = nc.dram_tensor("wv", [128, 4 * DIM], F8, kind="ExternalInput")
    rel_in = nc.dram_tensor("rel", [HD, 2 * 196], BF, kind="ExternalInput")
    kpat_in = nc.dram_tensor("kpat", [28, 392], BF, kind="ExternalInput")
    wp_in = nc.dram_tensor("wp", [128, 4 * DIM], F8, kind="ExternalInput")
    w1_in = nc.dram_tensor("w1", [128, 4 * MLP], F8, kind="ExternalInput")
    w2_in = nc.dram_tensor("w2", [128, 12 * DIM], F8, kind="ExternalInput")
    b2_in = nc.dram_tensor("b2", [DIM], F8, kind="ExternalInput")
    out_d = nc.dram_tensor("out", [NVAL, DIM], F32, kind="ExternalOutput")

    # k operand in DRAM: rows 0:64 x [NH, NTOK] bf16, written in phase B.
    kT_d = nc.dram_tensor("kT_d", [64, NH * NTOK], BF)
    y_d = nc.dram_tensor("y_d", [HP * HP, DIM], BF)

    with tile.TileContext(nc) as tc:
      with tc.tile_pool(name="singles", bufs=1) as singles:
        ident_f = singles.tile([128, 128], F32)
        make_identity(nc, ident_f[:])
        identB = singles.tile([128, 128], BF)
        nc.vector.tensor_copy(out=identB[:], in_=ident_f[:])

        eps_t = singles.tile([128, 1], F32)
        nc.vector.memset(eps_t[:], EPS)
        cb64 = singles.tile([1, 64], BF)
        nc.gpsimd.memset(cb64[:], 1.0)
        ones_f8 = singles.tile([1, 128], F8)
        nc.gpsimd.memset(ones_f8[:], 1.0)
        onesrow_f = singles.tile([1, 392], F32)
        nc.gpsimd.memset(onesrow_f[:], 1.0)
        magic_t = singles.tile([128, 4], F32)
        nc.vector.memset(magic_t[:], QMAGIC)
        c15_t = singles.tile([128, 4], F32)
        nc.vector.memset(c15_t[:], 1.5)

        # weights
        wqk_t = singles.tile([128, 4, 2 * DIM], F8)
        nc.sync.dma_start(out=wqk_t[:], in_=wqk_in.ap())
        wv_t = singles.tile([128, 4, DIM], F8)
        nc.sync.dma_start(out=wv_t[:], in_=wv_in.ap())
        relm_t = singles.tile([HD, 2 * 196], BF)
        nc.sync.dma_start(out=relm_t[:], in_=rel_in.ap())
        wp_t = singles.tile([128, 4, DIM], F8)
        nc.sync.dma_start(out=wp_t[:], in_=wp_in.ap())
        w1_t = singles.tile([128, 4, MLP], F8)
        nc.sync.dma_start(out=w1_t[:], in_=w1_in.ap())
        w2_t = singles.tile([128, 12, DIM], F8)
        nc.sync.dma_start(out=w2_t[:], in_=w2_in.ap())
        b2row = singles.tile([1, DIM], F8)
        nc.sync.dma_start(out=b2row[:], in_=b2_in.ap())

        with tc.tile_pool(name="attops", bufs=1) as attops:
          qb = [attops.tile([KR, NTOK], BF, name=f"qb{h}") for h in range(NH)]
          vw = [attops.tile([98, 2, NH * VS], F8, name=f"vw{w}") for w in range(NWIN)]
          for w in range(NWIN):
              e = [nc.vector, nc.gpsimd][w % 2]
              e.memset(_ap(vw[w], 64, [[NH * VS, 2], [VS, NH], [1, 1]]), 1.0)
          kta2 = [attops.tile([KR, NH, 392], BF, name=f"kta{i}") for i in range(2)]
          for i in range(2):
              nc.sync.dma_start(
                  out=kta2[i][64:KR, :, :],
                  in_=bass.AP(tensor=kpat_in.ap().tensor, offset=0,
                              ap=[[392, 28], [0, NH], [1, 392]]))

          with tc.tile_pool(name="pHT", bufs=1) as pHT:
            hT = pHT.tile([128, 4, NTOK], F8, name="hT")
            # chunk 3: row0 = 1.0 (bias row), rows 1:128 = 0; 3-way col split
            for i in range(4):
                e = [nc.vector, nc.gpsimd][i % 2]
                c0 = i * 1225
                e.memset(_ap(hT, 3 * NTOK + c0, [[1, 1225]], p=(0, 1)), 1.0)
                e.memset(_ap(hT, 3 * NTOK + c0, [[1, 1225]], p=(1, 128)), 0.0)
            # zero padding tokens in chunks 0:3 (right-edge and bottom windows)
            for c in range(3):
                e = [nc.vector, nc.gpsimd][c % 2]
                # right-edge windows w%5==4, cols 8:14 of each window row
                e.memset(_ap(hT, c * NTOK + 4 * 196 + 8, [[980, 5], [14, 14], [1, 6]]),
                         0.0)
                # bottom windows 20..24, rows 8:14
                e.memset(_ap(hT, c * NTOK + 20 * 196 + 8 * 14, [[196, 5], [1, 84]]),
                         0.0)

            # ===== Phase A: LN1 + transpose into window-major fp8 hT =====
            with tc.tile_pool(name="pA", bufs=4) as pA, \
                 tc.tile_pool(name="pA_ps", bufs=4, space="PSUM") as pA_ps:
                for ch in range(8):
                    xc = pA.tile([128, 4, DIM], F32, tag="xc")
                    nc.sync.dma_start(
                        out=xc[:],
                        in_=_dram_ap(x_in, 512 * ch * DIM,
                                     [[DIM, 128], [128 * DIM, 4], [1, DIM]]))
                    mvall = pA.tile([128, 4, 2], F32, tag="mva")
                    for tt in range(4):
                        stats = pA.tile([128, 6], F32, tag="st")
                        nc.vector.bn_stats(out=stats[:], in_=xc[:, tt, :])
                        nc.vector.bn_aggr(out=mvall[:, tt, :], in_=stats[:])
                    rstd = pA.tile([128, 4], F32, tag="rstd")
                    nc.scalar.activation(out=rstd[:], in_=_ap(mvall, 1, [[2, 4]]),
                                         func=AF.Sqrt, bias=eps_t[:], scale=1.0)
                    nc.vector.reciprocal(out=rstd[:], in_=rstd[:])
                    for tt in range(4):
                        t = 4 * ch + tt
                        nmr = pA.tile([128, 1], F32, tag="nmr")
                        nc.vector.scalar_tensor_tensor(out=nmr[:], in0=mvall[:, tt, 0:1],
                                                       scalar=-1.0, in1=rstd[:, tt:tt + 1],
                                                       op0=ALU.mult, op1=ALU.mult)
                        hn = pA.tile([128, DIM], BF, tag="hn")
                        nc.scalar.activation(out=hn[:], in_=xc[:, tt, :],
                                             func=AF.Identity, bias=nmr[:],
                                             scale=rstd[:, tt:tt + 1])
                        pt = pA_ps.tile([128, 3, 128], BF, tag="tr")
                        for c in range(3):
                            nc.tensor.transpose(pt[:, c, :], hn[:, c * 128:(c + 1) * 128],
                                                identB[:])
                        # window-major scatter: rows r0, r0+1 of the image
                        # (4 full col-windows of 14 + 1 partial of 8)
                        r0 = 2 * t
                        band, rb = r0 // 14, r0 % 14
                        e = [nc.gpsimd, nc.vector][t % 2]
                        dst = _ap(hT, band * 980 + rb * 14,
                                  [[NTOK, 3], [14, 2], [196, 4], [1, 14]])
                        src = bass.AP(tensor=pt[:].tensor, offset=pt[:].offset,
                                      ap=[pt[:].ap[0], [128, 3], [64, 2], [1, 56]])
                        e.tensor_copy(out=dst, in_=src)
                        dst = _ap(hT, band * 980 + 4 * 196 + rb * 14,
                                  [[NTOK, 3], [14, 2], [1, 8]])
                        src = bass.AP(tensor=pt[:].tensor, offset=pt[:].offset + 56,
                                      ap=[pt[:].ap[0], [128, 3], [64, 2], [1, 8]])
                        e.tensor_copy(out=dst, in_=src)

            # ===== Phase B: qk DR matmuls, rel, v =====
            with tc.tile_pool(name="pB", bufs=3) as pB, \
                 tc.tile_pool(name="pB_ps", bufs=3, space="PSUM") as pB_ps, \
                 tc.tile_pool(name="pC_ps", bufs=2, space="PSUM") as pC_ps, \
                 tc.tile_pool(name="pBv_ps", bufs=2, space="PSUM") as pBv_ps:
                copy_rr = [0]

                def scaled_copy(dst, src):
                    i = copy_rr[0]; copy_rr[0] += 1
                    if i % 3 == 0:
                        nc.scalar.activation(out=dst, in_=src, func=AF.Identity,
                                             bias=0.0, scale=FSI)
                    elif i % 3 == 1:
                        nc.vector.tensor_scalar(out=dst, in0=src, scalar1=FSI,
                                                scalar2=None, op0=ALU.mult)
                    else:
                        nc.gpsimd.tensor_scalar(out=dst, in0=src, scalar1=FSI,
                                                scalar2=None, op0=ALU.mult)

                def emit_v(s_):
                    w, half = s_ // 2, s_ % 2
                    ps = pBv_ps.tile([98, DIM], F32, tag="v", name="vps")
                    for j in range(2):
                        nc.tensor.matmul(ps[:],
                                         _ap(hT, 2 * j * NTOK + 98 * s_,
                                             [[NTOK, 2], [1, 98]]),
                                         wv_t[:, 2 * j:2 * j + 2, :],
                                         start=(j == 0), stop=(j == 1), perf_mode=DR)
                    scaled_copy(_ap(vw[w], half * NH * VS, [[VS, NH], [1, 64]]), ps[:])

                vnext = [0]
                relc = [0]
                for m in range(6):
                    for gi, (p0, plen) in enumerate(GROUPS):
                        ps = pB_ps.tile([128, 392], F32, tag="qk")
                        for j in range(2):
                            nc.tensor.matmul(
                                ps[:, 0:plen],
                                wqk_t[:, 2 * j:2 * j + 2, m * 128:(m + 1) * 128],
                                _ap(hT, 2 * j * NTOK + p0, [[NTOK, 2], [1, plen]]),
                                start=(j == 0), stop=(j == 1), perf_mode=DR)
                        if m < 3:
                            for half in range(2):
                                h = 2 * m + half
                                scaled_copy(qb[h][0:64, p0:p0 + plen],
                                            ps[64 * half:64 * half + 64, 0:plen])
                        else:
                            mm = m - 3
                            kst = pB.tile([128, 392], BF, tag="kst", bufs=4,
                                          name="kst")
                            scaled_copy(kst[:, 0:plen], ps[:, 0:plen])
                            nc.sync.dma_start(
                                out=_dram_ap(kT_d, 2 * mm * NTOK + p0,
                                             [[NTOK, 2], [NH * NTOK, 64], [1, plen]]),
                                in_=kst[:, 0:plen])
                        if vnext[0] < 50 and (m, gi) != (0, 0):
                            emit_v(vnext[0])
                            vnext[0] += 1
                    if m < 3:
                        # rel rows for heads 2m, 2m+1 (q complete for them now)
                        for half in range(2):
                            h = 2 * m + half
                            for r in range(14):
                                ps = pC_ps.tile([14, 350], F32, tag="rel")
                                nc.tensor.matmul(
                                    ps[:], relm_t[:, r * 14:(r + 1) * 14],
                                    _ap(qb[h], r * 14, [[196, 25], [1, 14]], p=(0, 64)),
                                    start=True, stop=True)
                                dst = _ap(qb[h], r * 14, [[196, 25], [1, 14]], p=(64, 78))
                                i = relc[0]; relc[0] += 1
                                e = [nc.scalar, nc.vector, nc.gpsimd][i % 3]
                                if e is nc.scalar:
                                    e.copy(out=dst, in_=ps[:])
                                else:
                                    e.tensor_copy(out=dst, in_=ps[:])
                            for cc in range(14):
                                ps = pC_ps.tile([14, 350], F32, tag="rel")
                                nc.tensor.matmul(
                                    ps[:], relm_t[:, 196 + cc * 14:196 + (cc + 1) * 14],
                                    _ap(qb[h], cc, [[196, 25], [14, 14]], p=(0, 64)),
                                    start=True, stop=True)
                                dst = _ap(qb[h], cc, [[196, 25], [14, 14]], p=(78, 92))
                                i = relc[0]; relc[0] += 1
                                e = [nc.scalar, nc.vector, nc.gpsimd][i % 3]
                                if e is nc.scalar:
                                    e.copy(out=dst, in_=ps[:])
                                else:
                                    e.tensor_copy(out=dst, in_=ps[:])

          # ===== Phase D: attention + proj (hT freed) — with E1 interleaved ===
          with tc.tile_pool(name="pE1p", bufs=1) as pE1p:
            zts = [pE1p.tile([128, DIM], BF, name=f"zts{t}") for t in range(32)]
            h2T = [pE1p.tile([128, 4, 512], F8, name=f"h2T{g}") for g in range(8)]
            for g in range(8):
                e = [nc.vector, nc.gpsimd][g % 2]
                e.memset(_ap(h2T[g], 3 * 512, [[1, 512]], p=(0, 1)), 1.0)
                e.memset(_ap(h2T[g], 3 * 512, [[1, 512]], p=(1, 128)), 0.0)

            with tc.tile_pool(name="pD", bufs=4) as pD, \
                 tc.tile_pool(name="pDet", bufs=16) as pDet, \
                 tc.tile_pool(name="pDa", bufs=2) as pDa, \
                 tc.tile_pool(name="pE1", bufs=3) as pE1, \
                 tc.tile_pool(name="pDs_ps", bufs=2, space="PSUM") as pDs_ps, \
                 tc.tile_pool(name="pDo_ps", bufs=2, space="PSUM") as pDo_ps, \
                 tc.tile_pool(name="pDb_ps", bufs=2, space="PSUM") as pDb_ps, \
                 tc.tile_pool(name="pDp_ps", bufs=1, space="PSUM") as pDp_ps, \
                 tc.tile_pool(name="pE1_ps", bufs=1, space="PSUM") as pE1_ps:

                def e1_group(g):
                    xc = pE1.tile([128, 4, DIM], F32, tag="xe", name="xc")
                    nc.sync.dma_start(
                        out=xc[:],
                        in_=_dram_ap(x_in, 512 * g * DIM,
                                     [[DIM, 128], [128 * DIM, 4], [1, DIM]]))
                    mvall = pE1.tile([128, 4, 2], F32, tag="mva_e", name="mvall")
                    for tt in range(4):
                        yc = pE1.tile([128, DIM], BF, tag="ye", name="yc")
                        nc.sync.dma_start(
                            out=yc[:],
                            in_=_dram_ap(y_d, (8 * g + 2 * tt) * HP * DIM,
                                         [[HP * DIM, 2], [DIM, 64], [1, DIM]]))
                        zt = zts[4 * g + tt]
                        e = [nc.gpsimd, nc.vector][tt % 2]
                        e.tensor_tensor(out=zt[:], in0=xc[:, tt, :], in1=yc[:],
                                        op=ALU.add)
                        stats = pE1.tile([128, 6], F32, tag="st_e", name="stats")
                        nc.vector.bn_stats(out=stats[:], in_=zt[:])
                        nc.vector.bn_aggr(out=mvall[:, tt, :], in_=stats[:])
                    # rstd = rsqrt(var + eps) via Quake + 2 Newton iters (DVE only)
                    vpe = pE1.tile([128, 4], F32, tag="vpe", name="vpe")
                    nc.vector.tensor_scalar(out=vpe[:], in0=_ap(mvall, 1, [[2, 4]]),
                                            scalar1=EPS, scalar2=None, op0=ALU.add)
                    yq = pE1.tile([128, 4], F32, tag="yq", name="yq")
                    sh = yq[:].bitcast(I32)
                    nc.vector.tensor_scalar(out=sh, in0=vpe[:].bitcast(I32),
                                            scalar1=1, scalar2=None,
                                            op0=ALU.arith_shift_right)
                    nc.vector.tensor_tensor(out=sh, in0=magic_t[:].bitcast(I32),
                                            in1=sh, op=ALU.subtract)
                    tq = pE1.tile([128, 4], F32, tag="tq", name="tq")
                    for _ in range(2):
                        nc.vector.tensor_tensor(out=tq[:], in0=vpe[:], in1=yq[:],
                                                op=ALU.mult)
                        nc.vector.tensor_tensor(out=tq[:], in0=tq[:], in1=yq[:],
                                                op=ALU.mult)
                        nc.vector.scalar_tensor_tensor(out=tq[:], in0=tq[:],
                                                       scalar=-0.5, in1=c15_t[:],
                                                       op0=ALU.mult, op1=ALU.add)
                        nc.vector.tensor_tensor(out=yq[:], in0=yq[:], in1=tq[:],
                                                op=ALU.mult)
                    for tt in range(4):
                        nmr = pE1.tile([128, 1], F32, tag="nmr_e", name="nmr")
                        nc.vector.scalar_tensor_tensor(out=nmr[:], in0=mvall[:, tt, 0:1],
                                                       scalar=-1.0, in1=yq[:, tt:tt + 1],
                                                       op0=ALU.mult, op1=ALU.mult)
                        hn = pE1.tile([128, DIM], BF, tag="hn_e", name="hn")
                        e = [nc.gpsimd, nc.vector][tt % 2]
                        e.tensor_scalar(out=hn[:], in0=zts[4 * g + tt][:],
                                        scalar1=nmr[:], scalar2=yq[:, tt:tt + 1],
                                        op0=ALU.add, op1=ALU.mult)
                        pt = pE1_ps.tile([128, 3, 128], BF, tag="htr", name="pt")
                        for c in range(3):
                            nc.tensor.transpose(pt[:, c, :], hn[:, c * 128:(c + 1) * 128],
                                                identB[:])
                        dst = _ap(h2T[g], tt * 128, [[512, 3], [1, 128]])
                        e = [nc.vector, nc.gpsimd][tt % 2]
                        e.tensor_copy(out=dst, in_=pt[:])

                drr = [0]
                for gi, (p0, plen) in enumerate(GROUPS):
                    nwin = plen // 196
                    kTa = kta2[gi % 2]
                    nc.sync.dma_start(
                        out=kTa[0:64, :, 0:plen],
                        in_=_dram_ap(kT_d, p0,
                                     [[NH * NTOK, 64], [NTOK, NH], [1, plen]]))
                    attnT = pDa.tile([128, 4, 392], F8, tag="attnT", name="attnT")
                    if gi < 2:
                        nc.gpsimd.memset(_ap(attnT, 3 * 392, [[1, 392]], p=(0, 1)), 1.0)
                        nc.gpsimd.memset(_ap(attnT, 3 * 392, [[1, 392]], p=(1, 128)), 0.0)
                    for h in range(NH):
                        ets = []
                        for i in range(nwin):
                            st = pDs_ps.tile([98, 2, 196], F32, tag="st")
                            for j in range(2):
                                nc.tensor.matmul(
                                    st[:, j, :],
                                    kTa[:, h, 196 * i + 98 * j:196 * i + 98 * j + 98],
                                    qb[h][:, p0 + 196 * i:p0 + 196 * i + 196],
                                    start=True, stop=True)
                            et = pDet.tile([98, 2, 196], F8, tag="et")
                            nc.scalar.activation(out=et[:], in_=st[:], func=AF.Exp,
                                                 bias=0.0, scale=1.0)
                            ets.append(et)
                        oT = pDo_ps.tile([VS, 2, 196], F32, tag="oT")
                        for i in range(nwin):
                            nc.tensor.matmul(
                                oT[:, i, :],
                                _ap(vw[2 * gi + i], h * VS,
                                    [[NH * VS, 2], [1, VS]], p=(0, 98)),
                                ets[i][:], start=True, stop=True, perf_mode=DR)
                        # rz = 1/z row (bf16), alternate DVE / Pool-divide
                        rz = pD.tile([1, 392], BF, tag="rz")
                        i = drr[0]; drr[0] += 1
                        if i % 2 == 0:
                            with nc.allow_low_precision(reason="1/z bf16 ok"):
                                nc.vector.reciprocal(out=rz[:, 0:196 * nwin],
                                                     in_=oT[64:65, 0:nwin, :])
                        else:
                            nc.gpsimd.tensor_tensor(out=rz[:, 0:196 * nwin],
                                                    in0=onesrow_f[:, 0:196 * nwin],
                                                    in1=oT[64:65, 0:nwin, :],
                                                    op=ALU.divide)
                        zb = pDb_ps.tile([64, 2, 196], F32, tag="zb")
                        nc.tensor.matmul(zb[:, 0:nwin, :], cb64[:],
                                         rz[:, 0:196 * nwin], start=True, stop=True)
                        dst = _ap(attnT, (h // 2) * 392, [[196, 2], [1, 196]],
                                  p=((h % 2) * 64, (h % 2) * 64 + 64))
                        if nwin == 1:
                            dst = _ap(attnT, (h // 2) * 392, [[1, 196]],
                                      p=((h % 2) * 64, (h % 2) * 64 + 64))
                            e = [nc.vector, nc.gpsimd][i % 2]
                            e.tensor_tensor(out=dst, in0=oT[0:64, 0, :],
                                            in1=zb[:, 0, :], op=ALU.mult)
                        else:
                            e = [nc.vector, nc.gpsimd][i % 2]
                            e.tensor_tensor(out=dst, in0=oT[0:64, :, :],
                                            in1=zb[:], op=ALU.mult)
                    for i in range(nwin):
                        w = 2 * gi + i
                        ysb = pD.tile([98, 2, DIM], BF, tag="ysb")
                        for jj in range(2):
                            pj = pDp_ps.tile([98, DIM], F32, tag="pj")
                            sl = 196 * i + 98 * jj
                            for j in range(2):
                                nc.tensor.matmul(pj[:],
                                                 attnT[:, 2 * j:2 * j + 2, sl:sl + 98],
                                                 wp_t[:, 2 * j:2 * j + 2, :],
                                                 start=(j == 0), stop=(j == 1),
                                                 perf_mode=DR)
                            k = drr[0]; drr[0] += 1
                            if k % 2 == 0:
                                nc.vector.tensor_scalar(out=ysb[:, jj, :], in0=pj[:],
                                                        scalar1=FSI, scalar2=None,
                                                        op0=ALU.mult)
                            else:
                                nc.gpsimd.tensor_scalar(out=ysb[:, jj, :], in0=pj[:],
                                                        scalar1=FSI, scalar2=None,
                                                        op0=ALU.mult)
                        wo = (w // 5) * 14 * HP + (w % 5) * 14
                        for jj in range(2):
                            e = [nc.scalar, nc.sync][jj]
                            e.dma_start(
                                out=_dram_ap(y_d, (wo + 7 * jj * HP) * DIM,
                                             [[HP * DIM, 7], [DIM, 14], [1, DIM]]),
                                in_=ysb[:, jj, :])
                    for g in E1_AFTER.get(gi, []):
                        e1_group(g)

            # ===== Phase E2: fc1 + gelu + fc2 (attention operands freed) =====
            with tc.tile_pool(name="pE2", bufs=3) as pE2, \
                 tc.tile_pool(name="pE2g", bufs=2) as pE2g, \
                 tc.tile_pool(name="pE2_ps", bufs=2, space="PSUM") as pE2_ps, \
                 tc.tile_pool(name="pE3_ps", bufs=2, space="PSUM") as pE3_ps:
                for g in range(8):
                    gt = []
                    for p in range(6):
                        gtp = pE2g.tile([128, 2, 512], F8, tag=f"g{p}", name=f"g{p}")
                        gt.append(gtp)
                    for m in range(12):
                        ps = pE2_ps.tile([128, 512], F32, tag="fc1", name="ps1")
                        for j in range(2):
                            nc.tensor.matmul(ps[:],
                                             w1_t[:, 2 * j:2 * j + 2, m * 128:(m + 1) * 128],
                                             h2T[g][:, 2 * j:2 * j + 2, :],
                                             start=(j == 0), stop=(j == 1),
                                             perf_mode=DR)
                        nc.scalar.activation(out=gt[m // 2][:, m % 2, :], in_=ps[:],
                                             func=AF.Gelu, bias=0.0, scale=FSI)
                    ot = pE2.tile([128, 4, DIM], F32, tag="oe", name="ot")
                    for tt in range(4):
                        ps = pE3_ps.tile([128, DIM], F32, tag="fc2", name="ps2")
                        for p in range(6):
                            nc.tensor.matmul(ps[:], gt[p][:, :, tt * 128:(tt + 1) * 128],
                                             w2_t[:, 2 * p:2 * p + 2, :],
                                             start=(p == 0), stop=False, perf_mode=DR)
                        nc.tensor.matmul(ps[:], ones_f8[:], b2row[:],
                                         start=False, stop=True)
                        e = [nc.vector, nc.gpsimd][tt % 2]
                        e.scalar_tensor_tensor(out=ot[:, tt, :], in0=ps[:],
                                               scalar=FSI, in1=zts[4 * g + tt][:],
                                               op0=ALU.mult, op1=ALU.add)
                    nc.sync.dma_start(
                        out=_dram_ap(out_d, 512 * g * DIM,
                                     [[DIM, 128], [128 * DIM, 4], [1, DIM]]),
                        in_=ot[:])

    nc.compile()
    return nc


_NC = None


def _get_nc():
    global _NC
    if _NC is None:
        _NC = build_bass()
    return _NC


def _f8(a):
    return np.ascontiguousarray(
        np.clip(np.asarray(a, np.float32), -240.0, 240.0)).astype(
            ml_dtypes.float8_e4m3)


def _host_prep(inputs):
    f = np.float32
    bf = ml_dtypes.bfloat16
    ln1_w = np.asarray(inputs["ln1_w"], f); ln1_b = np.asarray(inputs["ln1_b"], f)
    qkv_w = np.asarray(inputs["qkv_w"], f); qkv_b = np.asarray(inputs["qkv_b"], f)
    proj_w = np.asarray(inputs["proj_w"], f); proj_b = np.asarray(inputs["proj_b"], f)
    ln2_w = np.asarray(inputs["ln2_w"], f); ln2_b = np.asarray(inputs["ln2_b"], f)
    fc1_w = np.asarray(inputs["fc1_w"], f); fc1_b = np.asarray(inputs["fc1_b"], f)
    fc2_w = np.asarray(inputs["fc2_w"], f); fc2_b = np.asarray(inputs["fc2_b"], f)
    rel_h = np.asarray(inputs["rel_pos_h"], f); rel_w = np.asarray(inputs["rel_pos_w"], f)

    wqk = (ln1_w[:, None] * qkv_w[:, :768]).copy()
    bqk = (ln1_b @ qkv_w[:, :768] + qkv_b[:768]).copy()
    wqk[:, :384] *= SCALE
    bqk[:384] *= SCALE
    wv = (ln1_w[:, None] * qkv_w[:, 768:]).copy()
    bv = ln1_b @ qkv_w[:, 768:] + qkv_b[768:]

    def chunk4(wmat, n, bias_row):
        # [384, n] -> [128, 4, n]: chunks 0..2 = w rows, chunk3 row0 = bias
        out = np.zeros((128, 4, n), f)
        for kc in range(3):
            out[:, kc, :] = wmat[kc * 128:(kc + 1) * 128, :]
        out[0, 3, :] = bias_row
        return out * FS

    wqk4 = chunk4(wqk, 768, np.concatenate([bqk[:384], np.zeros(384, f)]))
    wv4 = chunk4(wv, 384, np.zeros(384, f))
    bp = proj_b + bv @ proj_w
    wp4 = chunk4(proj_w, 384, bp)
    w1m = ln2_w[:, None] * fc1_w
    b1 = ln2_b @ fc1_w + fc1_b
    w14 = chunk4(w1m, MLP, b1)
    w2m = np.zeros((128, 12, DIM), f)
    for kc in range(12):
        w2m[:, kc, :] = fc2_w[kc * 128:(kc + 1) * 128, :]
    w2m *= FS

    coords = np.arange(WS)[:, None] - np.arange(WS)[None, :] + (WS - 1)
    Rh = rel_h[coords]
    Rw = rel_w[coords]
    rel = np.zeros((HD, 2 * 196), f)
    for r in range(14):
        rel[:, r * 14:(r + 1) * 14] = Rh[r].T / SCALE
    for c in range(14):
        rel[:, 196 + c * 14:196 + (c + 1) * 14] = Rw[c].T / SCALE

    kpat = np.zeros((28, 392), f)
    for j in range(14):
        for a in range(2):
            kpat[j, 196 * a + 14 * j:196 * a + 14 * j + 14] = 1.0
            kpat[14 + j, 196 * a + j::14][:14] = 1.0

    return {
        "wqk": _f8(wqk4.reshape(128, -1)),
        "wv": _f8(wv4.reshape(128, -1)),
        "rel": rel.astype(bf),
        "kpat": kpat.astype(bf),
        "wp": _f8(wp4.reshape(128, -1)),
        "w1": _f8(w14.reshape(128, -1)),
        "w2": _f8(w2m.reshape(128, -1)),
        "b2": _f8(fc2_b * FS),
    }


def kernel(**inputs):
    nc = _get_nc()
    shared = _host_prep(inputs)
    x = np.asarray(inputs["x"], np.float32).reshape(B, NVAL, DIM)
    in_maps = [dict(shared, x=np.ascontiguousarray(x[c])) for c in range(B)]
    res = run_bass_kernel_spmd(nc, in_maps, list(range(B)))
    out = np.stack([res.results[c]["out"] for c in range(B)])
    return out.reshape(B, H, W, DIM)


if __name__ == "__main__":
    build_bass()
    print("build ok")


# revision 18
# speedup vs baseline: 1.1269x; 1.0225x over previous
"""Bass/Trainium2 kernel v3 for nn_BlockForNormalWindow (windowed-attention
transformer block), data-parallel over batch across 8 NeuronCores.

v3 over v2: fp8e4 DoubleRow matmuls for qkv/v/proj/fc1/fc2 (weights x64,
biases folded via ones-row in a 4th K-chunk), window-major fp8 hT
(contiguous group slices, no hstage), bf16 score path with K=92 layout
(no garbage rows), fp8 ets/v with DoubleRow AV, PE-broadcast of 1/z,
Quake rsqrt on DVE for LN2 (no act-table thrash), 3-way engine rotation
for PSUM->SBUF copies, split E1/E2 MLP phase."""
import sys
sys.path.insert(0, '/opt/trn_rl_repo')

import numpy as np
import ml_dtypes
import concourse.bass as bass
import concourse.mybir as mybir
import concourse.tile as tile
from concourse import bacc
from concourse.bass_utils import run_bass_kernel_spmd
from concourse.masks import make_identity

F32 = mybir.dt.float32
I32 = mybir.dt.int32
BF = mybir.dt.bfloat16
F8 = mybir.dt.float8e4
AF = mybir.ActivationFunctionType
ALU = mybir.AluOpType
DR = mybir.MatmulPerfMode.DoubleRow

B, H, W = 8, 64, 64
DIM, NH, WS = 384, 6, 14
HD = DIM // NH
MLP = 4 * DIM
EPS = 1e-5
SCALE = HD ** -0.5
HP = 70
NWIN = 25
NTOK = NWIN * WS * WS        # 4900
NVAL = H * W                 # 4096
VS = 65                      # per-head stride in v layout (64 vals + ones col)
KR = 92                      # rows in k/q operand: q/k 0:64, relh 64:78, relw 78:92
FS = 64.0                    # fp8 weight pre-scale
FSI = 1.0 / FS
QMAGIC = 1.3211836172961055e+19   # 0x5f3759df as float32

GROUPS = [(g * 392, 392) for g in range(12)] + [(4704, 196)]
# E1 group g (image rows 8g:8g+8) ready after this D group index
E1_AFTER = {2: [0], 4: [1, 2], 7: [3, 4], 9: [5, 6], 12: [7]}


def _ap(t, offset_elems, dims, p=None):
    a = t[:, 0:1] if p is None else t[p[0]:p[1], 0:1]
    return bass.AP(tensor=a.tensor, offset=a.offset + offset_elems,
                   ap=[a.ap[0]] + dims)


def _dram_ap(t, offset_elems, dims):
    a = t.ap()
    return bass.AP(tensor=a.tensor, offset=offset_elems, ap=dims)


def build_bass():
    nc = bacc.Bacc("TRN2", target_bir_lowering=False, debug=False)

    x_in = nc.dram_tensor("x", [NVAL, DIM], F32, kind="ExternalInput")
    wqk_in = nc.dram_tensor("wqk", [128, 4 * 2 * DIM], F8, kind="ExternalInput")
    wv_in = nc.dram_tensor("wv", [128, 4 * DIM], F8, kind="ExternalInput")
    rel_in = nc.dram_tensor("rel", [HD, 2 * 196], BF, kind="ExternalInput")
    kpat_in = nc.dram_tensor("kpat", [28, 392], BF, kind="ExternalInput")
    wp_in = nc.dram_tensor("wp", [128, 4 * DIM], F8, kind="ExternalInput")
    w1_in = nc.dram_tensor("w1", [128, 4 * MLP], F8, kind="ExternalInput")
    w2_in = nc.dram_tensor("w2", [128, 12 * DIM], F8, kind="ExternalInput")
    b2_in = nc.dram_tensor("b2", [DIM], F8, kind="ExternalInput")
    out_d = nc.dram_tensor("out", [NVAL, DIM], F32, kind="ExternalOutput")

    # k operand in DRAM: rows 0:64 x [NH, NTOK] bf16, written in phase B.
    kT_d = nc.dram_tensor("kT_d", [64, NH * NTOK], BF)
    y_d = nc.dram_tensor("y_d", [HP * HP, DIM], BF)

    with tile.TileContext(nc) as tc:
      with tc.tile_pool(name="singles", bufs=1) as singles:
        ident_f = singles.tile([128, 128], F32)
        make_identity(nc, ident_f[:])
        identB = singles.tile([128, 128], BF)
        nc.vector.tensor_copy(out=identB[:], in_=ident_f[:])

        eps_t = singles.tile([128, 1], F32)
        nc.vector.memset(eps_t[:], EPS)
        cb64 = singles.tile([1, 64], BF)
        nc.gpsimd.memset(cb64[:], 1.0)
        ones_f8 = singles.tile([1, 128], F8)
        nc.gpsimd.memset(ones_f8[:], 1.0)
        onesrow_f = singles.tile([1, 392], F32)
        nc.gpsimd.memset(onesrow_f[:], 1.0)
        magic_t = singles.tile([128, 4], F32)
        nc.vector.memset(magic_t[:], QMAGIC)
        c15_t = singles.tile([128, 4], F32)
        nc.vector.memset(c15_t[:], 1.5)

        # weights
        wqk_t = singles.tile([128, 4, 2 * DIM], F8)
        nc.sync.dma_start(out=wqk_t[:], in_=wqk_in.ap())
        wv_t = singles.tile([128, 4, DIM], F8)
        nc.sync.dma_start(out=wv_t[:], in_=wv_in.ap())
        relm_t = singles.tile([HD, 2 * 196], BF)
        nc.sync.dma_start(out=relm_t[:], in_=rel_in.ap())
        wp_t = singles.tile([128, 4, DIM], F8)
        nc.sync.dma_start(out=wp_t[:], in_=wp_in.ap())
        w1_t = singles.tile([128, 4, MLP], F8)
        nc.sync.dma_start(out=w1_t[:], in_=w1_in.ap())
        w2_t = singles.tile([128, 12, DIM], F8)
        nc.sync.dma_start(out=w2_t[:], in_=w2_in.ap())
        b2row = singles.tile([1, DIM], F8)
        nc.sync.dma_start(out=b2row[:], in_=b2_in.ap())

        with tc.tile_pool(name="attops", bufs=1) as attops:
          qb = [attops.tile([KR, NTOK], BF, name=f"qb{h}") for h in range(NH)]
          vw = [attops.tile([98, 2, NH * VS], F8, name=f"vw{w}") for w in range(NWIN)]
          for w in range(NWIN):
              e = [nc.vector, nc.gpsimd][w % 2]
              e.memset(_ap(vw[w], 64, [[NH * VS, 2], [VS, NH], [1, 1]]), 1.0)
          kta2 = [attops.tile([KR, NH, 392], BF, name=f"kta{i}") for i in range(2)]
          for i in range(2):
              nc.sync.dma_start(
                  out=kta2[i][64:KR, :, :],
                  in_=bass.AP(tensor=kpat_in.ap().tensor, offset=0,
                              ap=[[392, 28], [0, NH], [1, 392]]))

          with tc.tile_pool(name="pHT", bufs=1) as pHT:
            hT = pHT.tile([128, 4, NTOK], F8, name="hT")
            # chunk 3: row0 = 1.0 (bias row), rows 1:128 = 0; 3-way col split
            for i in range(4):
                e = [nc.vector, nc.gpsimd][i % 2]
                c0 = i * 1225
                e.memset(_ap(hT, 3 * NTOK + c0, [[1, 1225]], p=(0, 1)), 1.0)
                e.memset(_ap(hT, 3 * NTOK + c0, [[1, 1225]], p=(1, 128)), 0.0)
            # zero padding tokens in chunks 0:3 (right-edge and bottom windows)
            for c in range(3):
                e = [nc.vector, nc.gpsimd][c % 2]
                # right-edge windows w%5==4, cols 8:14 of each window row
                e.memset(_ap(hT, c * NTOK + 4 * 196 + 8, [[980, 5], [14, 14], [1, 6]]),
                         0.0)
                # bottom windows 20..24, rows 8:14
                e.memset(_ap(hT, c * NTOK + 20 * 196 + 8 * 14, [[196, 5], [1, 84]]),
                         0.0)

            # ===== Phase A: LN1 + transpose into window-major fp8 hT =====
            with tc.tile_pool(name="pA", bufs=4) as pA, \
                 tc.tile_pool(name="pA_ps", bufs=4, space="PSUM") as pA_ps:
                for ch in range(8):
                    xc = pA.tile([128, 4, DIM], F32, tag="xc")
                    nc.sync.dma_start(
                        out=xc[:],
                        in_=_dram_ap(x_in, 512 * ch * DIM,
                                     [[DIM, 128], [128 * DIM, 4], [1, DIM]]))
                    mvall = pA.tile([128, 4, 2], F32, tag="mva")
                    for tt in range(4):
                        stats = pA.tile([128, 6], F32, tag="st")
                        nc.vector.bn_stats(out=stats[:], in_=xc[:, tt, :])
                        nc.vector.bn_aggr(out=mvall[:, tt, :], in_=stats[:])
                    rstd = pA.tile([128, 4], F32, tag="rstd")
                    nc.scalar.activation(out=rstd[:], in_=_ap(mvall, 1, [[2, 4]]),
                                         func=AF.Sqrt, bias=eps_t[:], scale=1.0)
                    nc.vector.reciprocal(out=rstd[:], in_=rstd[:])
                    for tt in range(4):
                        t = 4 * ch + tt
                        nmr = pA.tile([128, 1], F32, tag="nmr")
                        nc.vector.scalar_tensor_tensor(out=nmr[:], in0=mvall[:, tt, 0:1],
                                                       scalar=-1.0, in1=rstd[:, tt:tt + 1],
                                                       op0=ALU.mult, op1=ALU.mult)
                        hn = pA.tile([128, DIM], BF, tag="hn")
                        nc.scalar.activation(out=hn[:], in_=xc[:, tt, :],
                                             func=AF.Identity, bias=nmr[:],
                                             scale=rstd[:, tt:tt + 1])
                        pt = pA_ps.tile([128, 3, 128], BF, tag="tr")
                        for c in range(3):
                            nc.tensor.transpose(pt[:, c, :], hn[:, c * 128:(c + 1) * 128],
                                                identB[:])
                        # window-major scatter: rows r0, r0+1 of the image
                        # (4 full col-windows of 14 + 1 partial of 8)
                        r0 = 2 * t
                        band, rb = r0 // 14, r0 % 14
                        e = [nc.gpsimd, nc.vector][t % 2]
                        dst = _ap(hT, band * 980 + rb * 14,
                                  [[NTOK, 3], [14, 2], [196, 4], [1, 14]])
                        src = bass.AP(tensor=pt[:].tensor, offset=pt[:].offset,
                                      ap=[pt[:].ap[0], [128, 3], [64, 2], [1, 56]])
                        e.tensor_copy(out=dst, in_=src)
                        dst = _ap(hT, band * 980 + 4 * 196 + rb * 14,
                                  [[NTOK, 3], [14, 2], [1, 8]])
                        src = bass.AP(tensor=pt[:].tensor, offset=pt[:].offset + 56,
                                      ap=[pt[:].ap[0], [128, 3], [64, 2], [1, 8]])
                        e.tensor_copy(out=dst, in_=src)

            # ===== Phase B: qk DR matmuls, rel, v =====
            with tc.tile_pool(name="pB", bufs=3) as pB, \
                 tc.tile_pool(name="pB_ps", bufs=3, space="PSUM") as pB_ps, \
                 tc.tile_pool(name="pC_ps", bufs=2, space="PSUM") as pC_ps, \
                 tc.tile_pool(name="pBv_ps", bufs=2, space="PSUM") as pBv_ps:
                copy_rr = [0]

                def scaled_copy(dst, src):
                    i = copy_rr[0]; copy_rr[0] += 1
                    if i % 3 == 0:
                        nc.scalar.activation(out=dst, in_=src, func=AF.Identity,
                                             bias=0.0, scale=FSI)
                    elif i % 3 == 1:
                        nc.vector.tensor_scalar(out=dst, in0=src, scalar1=FSI,
                                                scalar2=None, op0=ALU.mult)
                    else:
                        nc.gpsimd.tensor_scalar(out=dst, in0=src, scalar1=FSI,
                                                scalar2=None, op0=ALU.mult)

                def emit_v(s_):
                    w, half = s_ // 2, s_ % 2
                    ps = pBv_ps.tile([98, DIM], F32, tag="v", name="vps")
                    for j in range(2):
                        nc.tensor.matmul(ps[:],
                                         _ap(hT, 2 * j * NTOK + 98 * s_,
                                             [[NTOK, 2], [1, 98]]),
                                         wv_t[:, 2 * j:2 * j + 2, :],
                                         start=(j == 0), stop=(j == 1), perf_mode=DR)
                    scaled_copy(_ap(vw[w], half * NH * VS, [[VS, NH], [1, 64]]), ps[:])

                vnext = [0]
                relc = [0]
                for m in range(6):
                    for gi, (p0, plen) in enumerate(GROUPS):
                        ps = pB_ps.tile([128, 392], F32, tag="qk")
                        for j in range(2):
                            nc.tensor.matmul(
                                ps[:, 0:plen],
                                wqk_t[:, 2 * j:2 * j + 2, m * 128:(m + 1) * 128],
                                _ap(hT, 2 * j * NTOK + p0, [[NTOK, 2], [1, plen]]),
                                start=(j == 0), stop=(j == 1), perf_mode=DR)
                        if m < 3:
                            for half in range(2):
                                h = 2 * m + half
                                scaled_copy(qb[h][0:64, p0:p0 + plen],
                                            ps[64 * half:64 * half + 64, 0:plen])
                        else:
                            mm = m - 3
                            kst = pB.tile([128, 392], BF, tag="kst", bufs=4,
                                          name="kst")
                            scaled_copy(kst[:, 0:plen], ps[:, 0:plen])
                            nc.sync.dma_start(
                                out=_dram_ap(kT_d, 2 * mm * NTOK + p0,
                                             [[NTOK, 2], [NH * NTOK, 64], [1, plen]]),
                                in_=kst[:, 0:plen])
                        if vnext[0] < 50 and (m, gi) != (0, 0):
                            emit_v(vnext[0])
                            vnext[0] += 1
                    if m < 3:
                        # rel rows for heads 2m, 2m+1 (q complete for them now)
                        for half in range(2):
                            h = 2 * m + half
                            for r in range(14):
                                ps = pC_ps.tile([14, 350], F32, tag="rel")
                                nc.tensor.matmul(
                                    ps[:], relm_t[:, r * 14:(r + 1) * 14],
                                    _ap(qb[h], r * 14, [[196, 25], [1, 14]], p=(0, 64)),
                                    start=True, stop=True)
                                dst = _ap(qb[h], r * 14, [[196, 25], [1, 14]], p=(64, 78))
                                i = relc[0]; relc[0] += 1
                                e = [nc.scalar, nc.vector, nc.gpsimd][i % 3]
                                if e is nc.scalar:
                                    e.copy(out=dst, in_=ps[:])
                                else:
                                    e.tensor_copy(out=dst, in_=ps[:])
                            for cc in range(14):
                                ps = pC_ps.tile([14, 350], F32, tag="rel")
                                nc.tensor.matmul(
                                    ps[:], relm_t[:, 196 + cc * 14:196 + (cc + 1) * 14],
                                    _ap(qb[h], cc, [[196, 25], [14, 14]], p=(0, 64)),
                                    start=True, stop=True)
                                dst = _ap(qb[h], cc, [[196, 25], [14, 14]], p=(78, 92))
                                i = relc[0]; relc[0] += 1
                                e = [nc.scalar, nc.vector, nc.gpsimd][i % 3]
                                if e is nc.scalar:
                                    e.copy(out=dst, in_=ps[:])
                                else:
                                    e.tensor_copy(out=dst, in_=ps[:])

          # ===== Phase D: attention + proj (hT freed) — with E1 interleaved ===
          with tc.tile_pool(name="pE1p", bufs=1) as pE1p:
            zts = [pE1p.tile([128, DIM], BF, name=f"zts{t}") for t in range(32)]
            hns = [pE1p.tile([128, DIM], BF, name=f"hns{t}") for t in range(32)]

            with tc.tile_pool(name="pD", bufs=6) as pD, \
                 tc.tile_pool(name="pDet", bufs=16) as pDet, \
                 tc.tile_pool(name="pDa", bufs=2) as pDa, \
                 tc.tile_pool(name="pE1", bufs=3) as pE1, \
                 tc.tile_pool(name="pDs_ps", bufs=2, space="PSUM") as pDs_ps, \
                 tc.tile_pool(name="pDo_ps", bufs=3, space="PSUM") as pDo_ps, \
                 tc.tile_pool(name="pDb_ps", bufs=1, space="PSUM") as pDb_ps, \
                 tc.tile_pool(name="pDp_ps", bufs=2, space="PSUM") as pDp_ps:

                def e1_group(g):
                    xc = pE1.tile([128, 4, DIM], F32, tag="xe", name="xc")
                    nc.sync.dma_start(
                        out=xc[:],
                        in_=_dram_ap(x_in, 512 * g * DIM,
                                     [[DIM, 128], [128 * DIM, 4], [1, DIM]]))
                    mvall = pE1.tile([128, 4, 2], F32, tag="mva_e", name="mvall")
                    for tt in range(4):
                        yc = pE1.tile([128, DIM], BF, tag="ye", name="yc")
                        nc.sync.dma_start(
                            out=yc[:],
                            in_=_dram_ap(y_d, (8 * g + 2 * tt) * HP * DIM,
                                         [[HP * DIM, 2], [DIM, 64], [1, DIM]]))
                        zt = zts[4 * g + tt]
                        e = [nc.gpsimd, nc.vector][tt % 2]
                        e.tensor_tensor(out=zt[:], in0=xc[:, tt, :], in1=yc[:],
                                        op=ALU.add)
                        stats = pE1.tile([128, 6], F32, tag="st_e", name="stats")
                        nc.vector.bn_stats(out=stats[:], in_=zt[:])
                        nc.vector.bn_aggr(out=mvall[:, tt, :], in_=stats[:])
                    # rstd = rsqrt(var + eps) via Quake + 2 Newton iters (DVE only)
                    vpe = pE1.tile([128, 4], F32, tag="vpe", name="vpe")
                    nc.vector.tensor_scalar(out=vpe[:], in0=_ap(mvall, 1, [[2, 4]]),
                                            scalar1=EPS, scalar2=None, op0=ALU.add)
                    yq = pE1.tile([128, 4], F32, tag="yq", name="yq")
                    sh = yq[:].bitcast(I32)
                    nc.vector.tensor_scalar(out=sh, in0=vpe[:].bitcast(I32),
                                            scalar1=1, scalar2=None,
                                            op0=ALU.arith_shift_right)
                    nc.vector.tensor_tensor(out=sh, in0=magic_t[:].bitcast(I32),
                                            in1=sh, op=ALU.subtract)
                    tq = pE1.tile([128, 4], F32, tag="tq", name="tq")
                    for _ in range(2):
                        nc.vector.tensor_tensor(out=tq[:], in0=vpe[:], in1=yq[:],
                                                op=ALU.mult)
                        nc.vector.tensor_tensor(out=tq[:], in0=tq[:], in1=yq[:],
                                                op=ALU.mult)
                        nc.vector.scalar_tensor_tensor(out=tq[:], in0=tq[:],
                                                       scalar=-0.5, in1=c15_t[:],
                                                       op0=ALU.mult, op1=ALU.add)
                        nc.vector.tensor_tensor(out=yq[:], in0=yq[:], in1=tq[:],
                                                op=ALU.mult)
                    for tt in range(4):
                        nmr = pE1.tile([128, 1], F32, tag="nmr_e", name="nmr")
                        nc.vector.scalar_tensor_tensor(out=nmr[:], in0=mvall[:, tt, 0:1],
                                                       scalar=-1.0, in1=yq[:, tt:tt + 1],
                                                       op0=ALU.mult, op1=ALU.mult)
                        e = [nc.gpsimd, nc.vector][tt % 2]
                        e.tensor_scalar(out=hns[4 * g + tt][:], in0=zts[4 * g + tt][:],
                                        scalar1=nmr[:], scalar2=yq[:, tt:tt + 1],
                                        op0=ALU.add, op1=ALU.mult)

                drr = [0]
                for gi, (p0, plen) in enumerate(GROUPS):
                    nwin = plen // 196
                    kTa = kta2[gi % 2]
                    nc.sync.dma_start(
                        out=kTa[0:64, :, 0:plen],
                        in_=_dram_ap(kT_d, p0,
                                     [[NH * NTOK, 64], [NTOK, NH], [1, plen]]))
                    attnT = pDa.tile([128, 4, 392], F8, tag="attnT", name="attnT")
                    if gi < 2:
                        nc.gpsimd.memset(_ap(attnT, 3 * 392, [[1, 392]], p=(0, 1)), 1.0)
                        nc.gpsimd.memset(_ap(attnT, 3 * 392, [[1, 392]], p=(1, 128)), 0.0)
                    for h0 in (0, 3):
                        hb = range(h0, h0 + 3)
                        ets = {}
                        for h in hb:
                            for i in range(nwin):
                                st = pDs_ps.tile([98, 2, 196], F32, tag="st")
                                for j in range(2):
                                    nc.tensor.matmul(
                                        st[:, j, :],
                                        kTa[:, h, 196 * i + 98 * j:196 * i + 98 * j + 98],
                                        qb[h][:, p0 + 196 * i:p0 + 196 * i + 196],
                                        start=True, stop=True)
                                et = pDet.tile([98, 2, 196], F8, tag="et")
                                nc.scalar.activation(out=et[:], in_=st[:], func=AF.Exp,
                                                     bias=0.0, scale=1.0)
                                ets[(h, i)] = et
                        oTs = {}
                        for h in hb:
                            oT = pDo_ps.tile([VS, 2, 196], F32, tag="oT")
                            for i in range(nwin):
                                nc.tensor.matmul(
                                    oT[:, i, :],
                                    _ap(vw[2 * gi + i], h * VS,
                                        [[NH * VS, 2], [1, VS]], p=(0, 98)),
                                    ets[(h, i)][:], start=True, stop=True,
                                    perf_mode=DR)
                            oTs[h] = oT
                        rzs = {}
                        for h in hb:
                            rz = pD.tile([1, 392], BF, tag="rz")
                            i = drr[0]; drr[0] += 1
                            if i % 2 == 0:
                                with nc.allow_low_precision(reason="1/z bf16 ok"):
                                    nc.vector.reciprocal(out=rz[:, 0:196 * nwin],
                                                         in_=oTs[h][64:65, 0:nwin, :])
                            else:
                                nc.gpsimd.tensor_tensor(out=rz[:, 0:196 * nwin],
                                                        in0=onesrow_f[:, 0:196 * nwin],
                                                        in1=oTs[h][64:65, 0:nwin, :],
                                                        op=ALU.divide)
                            rzs[h] = rz
                        for h in hb:
                            zb = pDb_ps.tile([64, 2, 196], F32, tag="zb")
                            nc.tensor.matmul(zb[:, 0:nwin, :], cb64[:],
                                             rzs[h][:, 0:196 * nwin],
                                             start=True, stop=True)
                            oT = oTs[h]
                            i = drr[0]; drr[0] += 1
                            e = [nc.vector, nc.gpsimd][i % 2]
                            if nwin == 1:
                                dst = _ap(attnT, (h // 2) * 392, [[1, 196]],
                                          p=((h % 2) * 64, (h % 2) * 64 + 64))
                                e.tensor_tensor(out=dst, in0=oT[0:64, 0, :],
                                                in1=zb[:, 0, :], op=ALU.mult)
                            else:
                                dst = _ap(attnT, (h // 2) * 392, [[196, 2], [1, 196]],
                                          p=((h % 2) * 64, (h % 2) * 64 + 64))
                                e.tensor_tensor(out=dst, in0=oT[0:64, :, :],
                                                in1=zb[:], op=ALU.mult)
                    for i in range(nwin):
                        w = 2 * gi + i
                        ysb = pD.tile([98, 2, DIM], BF, tag="ysb")
                        for jj in range(2):
                            pj = pDp_ps.tile([98, DIM], F32, tag="pj")
                            sl = 196 * i + 98 * jj
                            for j in range(2):
                                nc.tensor.matmul(pj[:],
                                                 attnT[:, 2 * j:2 * j + 2, sl:sl + 98],
                                                 wp_t[:, 2 * j:2 * j + 2, :],
                                                 start=(j == 0), stop=(j == 1),
                                                 perf_mode=DR)
                            k = drr[0]; drr[0] += 1
                            if k % 2 == 0:
                                nc.vector.tensor_scalar(out=ysb[:, jj, :], in0=pj[:],
                                                        scalar1=FSI, scalar2=None,
                                                        op0=ALU.mult)
                            else:
                                nc.gpsimd.tensor_scalar(out=ysb[:, jj, :], in0=pj[:],
                                                        scalar1=FSI, scalar2=None,
                                                        op0=ALU.mult)
                        wo = (w // 5) * 14 * HP + (w % 5) * 14
                        for jj in range(2):
                            e = [nc.scalar, nc.sync][jj]
                            e.dma_start(
                                out=_dram_ap(y_d, (wo + 7 * jj * HP) * DIM,
                                             [[HP * DIM, 7], [DIM, 14], [1, DIM]]),
                                in_=ysb[:, jj, :])
                    for g in E1_AFTER.get(gi, []):
                        e1_group(g)

            # ===== Phase E2: fc1 + gelu + fc2 (attention operands freed) =====
            with tc.tile_pool(name="pE2", bufs=3) as pE2, \
                 tc.tile_pool(name="pE2g", bufs=2) as pE2g, \
                 tc.tile_pool(name="pE2h", bufs=2) as pE2h, \
                 tc.tile_pool(name="pE2t_ps", bufs=2, space="PSUM") as pE2t_ps, \
                 tc.tile_pool(name="pE2_ps", bufs=2, space="PSUM") as pE2_ps, \
                 tc.tile_pool(name="pE3_ps", bufs=2, space="PSUM") as pE3_ps:
                for g in range(8):
                    h2T = pE2h.tile([128, 4, 512], F8, tag="h2T", name="h2T")
                    if g < 2:
                        e = [nc.vector, nc.gpsimd][g % 2]
                        e.memset(_ap(h2T, 3 * 512, [[1, 512]], p=(0, 1)), 1.0)
                        e.memset(_ap(h2T, 3 * 512, [[1, 512]], p=(1, 128)), 0.0)
                    for tt in range(4):
                        pt = pE2t_ps.tile([128, 3, 128], BF, tag="htr", name="pt")
                        hn = hns[4 * g + tt]
                        for c in range(3):
                            nc.tensor.transpose(pt[:, c, :], hn[:, c * 128:(c + 1) * 128],
                                                identB[:])
                        dst = _ap(h2T, tt * 128, [[512, 3], [1, 128]])
                        e = [nc.vector, nc.gpsimd][tt % 2]
                        e.tensor_copy(out=dst, in_=pt[:])
                    gt = [pE2g.tile([128, 2, 512], F8, tag=f"g{p}", name=f"g{p}")
                          for p in range(6)]
                    for m in range(12):
                        ps = pE2_ps.tile([128, 512], F32, tag="fc1", name="ps1")
                        for j in range(2):
                            nc.tensor.matmul(ps[:],
                                             w1_t[:, 2 * j:2 * j + 2, m * 128:(m + 1) * 128],
                                             h2T[:, 2 * j:2 * j + 2, :],
                                             start=(j == 0), stop=(j == 1),
                                             perf_mode=DR)
                        nc.scalar.activation(out=gt[m // 2][:, m % 2, :], in_=ps[:],
                                             func=AF.Gelu, bias=0.0, scale=FSI)
                    ot = pE2.tile([128, 4, DIM], F32, tag="oe", name="ot")
                    for tt in range(4):
                        ps = pE3_ps.tile([128, DIM], F32, tag="fc2", name="ps2")
                        for p in range(6):
                            nc.tensor.matmul(ps[:], gt[p][:, :, tt * 128:(tt + 1) * 128],
                                             w2_t[:, 2 * p:2 * p + 2, :],
                                             start=(p == 0), stop=False, perf_mode=DR)
                        nc.tensor.matmul(ps[:], ones_f8[:], b2row[:],
                                         start=False, stop=True)
                        e = [nc.vector, nc.gpsimd][tt % 2]
                        e.scalar_tensor_tensor(out=ot[:, tt, :], in0=ps[:],
                                               scalar=FSI, in1=zts[4 * g + tt][:],
                                               op0=ALU.mult, op1=ALU.add)
                    nc.sync.dma_start(
                        out=_dram_ap(out_d, 512 * g * DIM,
                                     [[DIM, 128], [128 * DIM, 4], [1, DIM]]),
                        in_=ot[:])

    nc.compile()
    return nc


_NC = None


def _get_nc():
    global _NC
    if _NC is None:
        _NC = build_bass()
    return _NC


def _f8(a):
    return np.ascontiguousarray(
        np.clip(np.asarray(a, np.float32), -240.0, 240.0)).astype(
            ml_dtypes.float8_e4m3)


def _host_prep(inputs):
    f = np.float32
    bf = ml_dtypes.bfloat16
    ln1_w = np.asarray(inputs["ln1_w"], f); ln1_b = np.asarray(inputs["ln1_b"], f)
    qkv_w = np.asarray(inputs["qkv_w"], f); qkv_b = np.asarray(inputs["qkv_b"], f)
    proj_w = np.asarray(inputs["proj_w"], f); proj_b = np.asarray(inputs["proj_b"], f)
    ln2_w = np.asarray(inputs["ln2_w"], f); ln2_b = np.asarray(inputs["ln2_b"], f)
    fc1_w = np.asarray(inputs["fc1_w"], f); fc1_b = np.asarray(inputs["fc1_b"], f)
    fc2_w = np.asarray(inputs["fc2_w"], f); fc2_b = np.asarray(inputs["fc2_b"], f)
    rel_h = np.asarray(inputs["rel_pos_h"], f); rel_w = np.asarray(inputs["rel_pos_w"], f)

    wqk = (ln1_w[:, None] * qkv_w[:, :768]).copy()
    bqk = (ln1_b @ qkv_w[:, :768] + qkv_b[:768]).copy()
    wqk[:, :384] *= SCALE
    bqk[:384] *= SCALE
    wv = (ln1_w[:, None] * qkv_w[:, 768:]).copy()
    bv = ln1_b @ qkv_w[:, 768:] + qkv_b[768:]

    def chunk4(wmat, n, bias_row):
        # [384, n] -> [128, 4, n]: chunks 0..2 = w rows, chunk3 row0 = bias
        out = np.zeros((128, 4, n), f)
        for kc in range(3):
            out[:, kc, :] = wmat[kc * 128:(kc + 1) * 128, :]
        out[0, 3, :] = bias_row
        return out * FS

    wqk4 = chunk4(wqk, 768, np.concatenate([bqk[:384], np.zeros(384, f)]))
    wv4 = chunk4(wv, 384, np.zeros(384, f))
    bp = proj_b + bv @ proj_w
    wp4 = chunk4(proj_w, 384, bp)
    w1m = ln2_w[:, None] * fc1_w
    b1 = ln2_b @ fc1_w + fc1_b
    w14 = chunk4(w1m, MLP, b1)
    w2m = np.zeros((128, 12, DIM), f)
    for kc in range(12):
        w2m[:, kc, :] = fc2_w[kc * 128:(kc + 1) * 128, :]
    w2m *= FS

    coords = np.arange(WS)[:, None] - np.arange(WS)[None, :] + (WS - 1)
    Rh = rel_h[coords]
    Rw = rel_w[coords]
    rel = np.zeros((HD, 2 * 196), f)
    for r in range(14):
        rel[:, r * 14:(r + 1) * 14] = Rh[r].T / SCALE
    for c in range(14):
        rel[:, 196 + c * 14:196 + (c + 1) * 14] = Rw[c].T / SCALE

    kpat = np.zeros((28, 392), f)
    for j in range(14):
        for a in range(2):
            kpat[j, 196 * a + 14 * j:196 * a + 14 * j + 14] = 1.0
            kpat[14 + j, 196 * a + j::14][:14] = 1.0

    return {
        "wqk": _f8(wqk4.reshape(128, -1)),
        "wv": _f8(wv4.reshape(128, -1)),
        "rel": rel.astype(bf),
        "kpat": kpat.astype(bf),
        "wp": _f8(wp4.reshape(128, -1)),
        "w1": _f8(w14.reshape(128, -1)),
        "w2": _f8(w2m.reshape(128, -1)),
        "b2": _f8(fc2_b * FS),
    }


def kernel(**inputs):
    nc = _get_nc()
    shared = _host_prep(inputs)
    x = np.asarray(inputs["x"], np.float32).reshape(B, NVAL, DIM)
    in_maps = [dict(shared, x=np.ascontiguousarray(x[c])) for c in range(B)]
    res = run_bass_kernel_spmd(nc, in_maps, list(range(B)))
    out = np.stack([res.results[c]["out"] for c in range(B)])
    return out.reshape(B, H, W, DIM)


if __name__ == "__main__":
    build_bass()
    print("build ok")


# revision 23
# speedup vs baseline: 1.2141x; 1.0774x over previous
"""Bass/Trainium2 kernel v3 for nn_BlockForNormalWindow (windowed-attention
transformer block), data-parallel over batch across 8 NeuronCores.

v3 over v2: fp8e4 DoubleRow matmuls for qkv/v/proj/fc1/fc2 (weights x64,
biases folded via ones-row in a 4th K-chunk), window-major fp8 hT
(contiguous group slices, no hstage), bf16 score path with K=92 layout
(no garbage rows), fp8 ets/v with DoubleRow AV, PE-broadcast of 1/z,
Quake rsqrt on DVE for LN2 (no act-table thrash), 3-way engine rotation
for PSUM->SBUF copies, split E1/E2 MLP phase."""
import sys
sys.path.insert(0, '/opt/trn_rl_repo')

import numpy as np
import ml_dtypes
import concourse.bass as bass
import concourse.mybir as mybir
import concourse.tile as tile
from concourse import bacc
from concourse.bass_utils import run_bass_kernel_spmd
from concourse.masks import make_identity

F32 = mybir.dt.float32
I32 = mybir.dt.int32
BF = mybir.dt.bfloat16
F8 = mybir.dt.float8e4
AF = mybir.ActivationFunctionType
ALU = mybir.AluOpType
DR = mybir.MatmulPerfMode.DoubleRow

B, H, W = 8, 64, 64
DIM, NH, WS = 384, 6, 14
HD = DIM // NH
MLP = 4 * DIM
EPS = 1e-5
SCALE = HD ** -0.5
HP = 70
NWIN = 25
NTOK = NWIN * WS * WS        # 4900
NVAL = H * W                 # 4096
VS = 65                      # per-head stride in v layout (64 vals + ones col)
KR = 92                      # rows in k/q operand: q/k 0:64, relh 64:78, relw 78:92
FS = 64.0                    # fp8 weight pre-scale
FSI = 1.0 / FS
QMAGIC = 1.3211836172961055e+19   # 0x5f3759df as float32

GROUPS = [(g * 392, 392) for g in range(12)] + [(4704, 196)]
# E1 group g (image rows 8g:8g+8) ready after this D group index
E1_AFTER = {2: [0], 4: [1, 2], 7: [3, 4], 9: [5, 6], 12: [7]}


def _ap(t, offset_elems, dims, p=None):
    a = t[:, 0:1] if p is None else t[p[0]:p[1], 0:1]
    return bass.AP(tensor=a.tensor, offset=a.offset + offset_elems,
                   ap=[a.ap[0]] + dims)


def _dram_ap(t, offset_elems, dims):
    a = t.ap()
    return bass.AP(tensor=a.tensor, offset=offset_elems, ap=dims)


def build_bass():
    nc = bacc.Bacc("TRN2", target_bir_lowering=False, debug=False)

    x_in = nc.dram_tensor("x", [NVAL, DIM], F32, kind="ExternalInput")
    wqk_in = nc.dram_tensor("wqk", [128, 4 * 2 * DIM], F8, kind="ExternalInput")
    wv_in = nc.dram_tensor("wv", [128, 4 * DIM], F8, kind="ExternalInput")
    rel_in = nc.dram_tensor("rel", [HD, 2 * 196], BF, kind="ExternalInput")
    kpat_in = nc.dram_tensor("kpat", [28, 392], BF, kind="ExternalInput")
    wp_in = nc.dram_tensor("wp", [128, 4 * DIM], F8, kind="ExternalInput")
    w1_in = nc.dram_tensor("w1", [128, 4 * MLP], F8, kind="ExternalInput")
    w2_in = nc.dram_tensor("w2", [128, 12 * DIM], F8, kind="ExternalInput")
    b2_in = nc.dram_tensor("b2", [DIM], F8, kind="ExternalInput")
    out_d = nc.dram_tensor("out", [NVAL, DIM], F32, kind="ExternalOutput")

    # k operand in DRAM: rows 0:64 x [NH, NTOK] bf16, written in phase B.
    kT_d = nc.dram_tensor("kT_d", [64, NH * NTOK], BF)
    y_d = nc.dram_tensor("y_d", [HP * HP, DIM], BF)

    with tile.TileContext(nc) as tc:
      with tc.tile_pool(name="singles", bufs=1) as singles:
        ident_f = singles.tile([128, 128], F32)
        make_identity(nc, ident_f[:])
        identB = singles.tile([128, 128], BF)
        nc.vector.tensor_copy(out=identB[:], in_=ident_f[:])

        eps_t = singles.tile([128, 1], F32)
        nc.vector.memset(eps_t[:], EPS)
        cb64 = singles.tile([1, 64], BF)
        nc.gpsimd.memset(cb64[:], 1.0)
        ones_f8 = singles.tile([1, 128], F8)
        nc.gpsimd.memset(ones_f8[:], 1.0)
        onesrow_f = singles.tile([1, 392], F32)
        nc.gpsimd.memset(onesrow_f[:], 1.0)
        magic_t = singles.tile([128, 4], F32)
        nc.vector.memset(magic_t[:], QMAGIC)
        c15_t = singles.tile([128, 4], F32)
        nc.vector.memset(c15_t[:], 1.5)

        # weights
        wqk_t = singles.tile([128, 4, 2 * DIM], F8)
        nc.sync.dma_start(out=wqk_t[:], in_=wqk_in.ap())
        wv_t = singles.tile([128, 4, DIM], F8)
        nc.sync.dma_start(out=wv_t[:], in_=wv_in.ap())
        relm_t = singles.tile([HD, 2 * 196], BF)
        nc.sync.dma_start(out=relm_t[:], in_=rel_in.ap())
        wp_t = singles.tile([128, 4, DIM], F8)
        nc.sync.dma_start(out=wp_t[:], in_=wp_in.ap())
        w1_t = singles.tile([128, 4, MLP], F8)
        nc.sync.dma_start(out=w1_t[:], in_=w1_in.ap())
        w2_t = singles.tile([128, 12, DIM], F8)
        nc.sync.dma_start(out=w2_t[:], in_=w2_in.ap())
        b2row = singles.tile([1, DIM], F8)
        nc.sync.dma_start(out=b2row[:], in_=b2_in.ap())

        with tc.tile_pool(name="attops", bufs=1) as attops:
          qb = [attops.tile([KR, NTOK], BF, name=f"qb{h}") for h in range(NH)]
          vw = [attops.tile([98, 2, NH * VS], F8, name=f"vw{w}") for w in range(NWIN)]
          for w in range(NWIN):
              e = [nc.vector, nc.gpsimd][w % 2]
              e.memset(_ap(vw[w], 64, [[NH * VS, 2], [VS, NH], [1, 1]]), 1.0)
          kta2 = [attops.tile([KR, NH, 392], BF, name=f"kta{i}") for i in range(2)]
          for i in range(2):
              nc.sync.dma_start(
                  out=kta2[i][64:KR, :, :],
                  in_=bass.AP(tensor=kpat_in.ap().tensor, offset=0,
                              ap=[[392, 28], [0, NH], [1, 392]]))

          with tc.tile_pool(name="pHT", bufs=1) as pHT:
            hT = pHT.tile([128, 4, NTOK], F8, name="hT")
            # chunk 3: row0 = 1.0 (bias row), rows 1:128 = 0; 3-way col split
            for i in range(4):
                e = [nc.vector, nc.gpsimd][i % 2]
                c0 = i * 1225
                e.memset(_ap(hT, 3 * NTOK + c0, [[1, 1225]], p=(0, 1)), 1.0)
                e.memset(_ap(hT, 3 * NTOK + c0, [[1, 1225]], p=(1, 128)), 0.0)
            # zero padding tokens in chunks 0:3 (right-edge and bottom windows)
            for c in range(3):
                e = [nc.vector, nc.gpsimd][c % 2]
                # right-edge windows w%5==4, cols 8:14 of each window row
                e.memset(_ap(hT, c * NTOK + 4 * 196 + 8, [[980, 5], [14, 14], [1, 6]]),
                         0.0)
                # bottom windows 20..24, rows 8:14
                e.memset(_ap(hT, c * NTOK + 20 * 196 + 8 * 14, [[196, 5], [1, 84]]),
                         0.0)

            # ===== Phase A: LN1 + transpose into window-major fp8 hT =====
            with tc.tile_pool(name="pA", bufs=4) as pA, \
                 tc.tile_pool(name="pA_ps", bufs=4, space="PSUM") as pA_ps:
                for ch in range(8):
                    xc = pA.tile([128, 4, DIM], F32, tag="xc")
                    nc.sync.dma_start(
                        out=xc[:],
                        in_=_dram_ap(x_in, 512 * ch * DIM,
                                     [[DIM, 128], [128 * DIM, 4], [1, DIM]]))
                    mvall = pA.tile([128, 4, 2], F32, tag="mva")
                    for tt in range(4):
                        stats = pA.tile([128, 6], F32, tag="st")
                        nc.vector.bn_stats(out=stats[:], in_=xc[:, tt, :])
                        nc.vector.bn_aggr(out=mvall[:, tt, :], in_=stats[:])
                    rstd = pA.tile([128, 4], F32, tag="rstd")
                    nc.scalar.activation(out=rstd[:], in_=_ap(mvall, 1, [[2, 4]]),
                                         func=AF.Sqrt, bias=eps_t[:], scale=1.0)
                    nc.vector.reciprocal(out=rstd[:], in_=rstd[:])
                    for tt in range(4):
                        t = 4 * ch + tt
                        nmr = pA.tile([128, 1], F32, tag="nmr")
                        nc.vector.scalar_tensor_tensor(out=nmr[:], in0=mvall[:, tt, 0:1],
                                                       scalar=-1.0, in1=rstd[:, tt:tt + 1],
                                                       op0=ALU.mult, op1=ALU.mult)
                        hn = pA.tile([128, DIM], BF, tag="hn")
                        nc.scalar.activation(out=hn[:], in_=xc[:, tt, :],
                                             func=AF.Identity, bias=nmr[:],
                                             scale=rstd[:, tt:tt + 1])
                        pt = pA_ps.tile([128, 3, 128], BF, tag="tr")
                        for c in range(3):
                            nc.tensor.transpose(pt[:, c, :], hn[:, c * 128:(c + 1) * 128],
                                                identB[:])
                        # window-major scatter: rows r0, r0+1 of the image
                        # (4 full col-windows of 14 + 1 partial of 8)
                        r0 = 2 * t
                        band, rb = r0 // 14, r0 % 14
                        e = [nc.gpsimd, nc.vector][t % 2]
                        dst = _ap(hT, band * 980 + rb * 14,
                                  [[NTOK, 3], [14, 2], [196, 4], [1, 14]])
                        src = bass.AP(tensor=pt[:].tensor, offset=pt[:].offset,
                                      ap=[pt[:].ap[0], [128, 3], [64, 2], [1, 56]])
                        e.tensor_copy(out=dst, in_=src)
                        dst = _ap(hT, band * 980 + 4 * 196 + rb * 14,
                                  [[NTOK, 3], [14, 2], [1, 8]])
                        src = bass.AP(tensor=pt[:].tensor, offset=pt[:].offset + 56,
                                      ap=[pt[:].ap[0], [128, 3], [64, 2], [1, 8]])
                        e.tensor_copy(out=dst, in_=src)

            # ===== Phase B: qk DR matmuls, rel, v =====
            with tc.tile_pool(name="pB", bufs=3) as pB, \
                 tc.tile_pool(name="pB_ps", bufs=3, space="PSUM") as pB_ps, \
                 tc.tile_pool(name="pC_ps", bufs=2, space="PSUM") as pC_ps, \
                 tc.tile_pool(name="pBv_ps", bufs=2, space="PSUM") as pBv_ps:
                copy_rr = [0]

                def scaled_copy(dst, src):
                    i = copy_rr[0]; copy_rr[0] += 1
                    if i % 3 == 0:
                        nc.scalar.activation(out=dst, in_=src, func=AF.Identity,
                                             bias=0.0, scale=FSI)
                    elif i % 3 == 1:
                        nc.vector.tensor_scalar(out=dst, in0=src, scalar1=FSI,
                                                scalar2=None, op0=ALU.mult)
                    else:
                        nc.gpsimd.tensor_scalar(out=dst, in0=src, scalar1=FSI,
                                                scalar2=None, op0=ALU.mult)

                def emit_v(s_):
                    w, half = s_ // 2, s_ % 2
                    ps = pBv_ps.tile([98, DIM], F32, tag="v", name="vps")
                    for j in range(2):
                        nc.tensor.matmul(ps[:],
                                         _ap(hT, 2 * j * NTOK + 98 * s_,
                                             [[NTOK, 2], [1, 98]]),
                                         wv_t[:, 2 * j:2 * j + 2, :],
                                         start=(j == 0), stop=(j == 1), perf_mode=DR)
                    scaled_copy(_ap(vw[w], half * NH * VS, [[VS, NH], [1, 64]]), ps[:])

                vnext = [0]
                relc = [0]
                for m in range(6):
                    for gi, (p0, plen) in enumerate(GROUPS):
                        ps = pB_ps.tile([128, 392], F32, tag="qk")
                        for j in range(2):
                            nc.tensor.matmul(
                                ps[:, 0:plen],
                                wqk_t[:, 2 * j:2 * j + 2, m * 128:(m + 1) * 128],
                                _ap(hT, 2 * j * NTOK + p0, [[NTOK, 2], [1, plen]]),
                                start=(j == 0), stop=(j == 1), perf_mode=DR)
                        if m < 3:
                            for half in range(2):
                                h = 2 * m + half
                                scaled_copy(qb[h][0:64, p0:p0 + plen],
                                            ps[64 * half:64 * half + 64, 0:plen])
                        else:
                            mm = m - 3
                            kst = pB.tile([128, 392], BF, tag="kst", bufs=4,
                                          name="kst")
                            scaled_copy(kst[:, 0:plen], ps[:, 0:plen])
                            nc.sync.dma_start(
                                out=_dram_ap(kT_d, 2 * mm * NTOK + p0,
                                             [[NTOK, 2], [NH * NTOK, 64], [1, plen]]),
                                in_=kst[:, 0:plen])
                        if vnext[0] < 50 and (m, gi) != (0, 0):
                            emit_v(vnext[0])
                            vnext[0] += 1
                    if m < 3:
                        # rel rows for heads 2m, 2m+1 (q complete for them now)
                        for half in range(2):
                            h = 2 * m + half
                            for r in range(14):
                                ps = pC_ps.tile([14, 350], F32, tag="rel")
                                nc.tensor.matmul(
                                    ps[:], relm_t[:, r * 14:(r + 1) * 14],
                                    _ap(qb[h], r * 14, [[196, 25], [1, 14]], p=(0, 64)),
                                    start=True, stop=True)
                                dst = _ap(qb[h], r * 14, [[196, 25], [1, 14]], p=(64, 78))
                                i = relc[0]; relc[0] += 1
                                e = [nc.scalar, nc.vector, nc.gpsimd][i % 3]
                                if e is nc.scalar:
                                    e.copy(out=dst, in_=ps[:])
                                else:
                                    e.tensor_copy(out=dst, in_=ps[:])
                            for cc in range(14):
                                ps = pC_ps.tile([14, 350], F32, tag="rel")
                                nc.tensor.matmul(
                                    ps[:], relm_t[:, 196 + cc * 14:196 + (cc + 1) * 14],
                                    _ap(qb[h], cc, [[196, 25], [14, 14]], p=(0, 64)),
                                    start=True, stop=True)
                                dst = _ap(qb[h], cc, [[196, 25], [14, 14]], p=(78, 92))
                                i = relc[0]; relc[0] += 1
                                e = [nc.scalar, nc.vector, nc.gpsimd][i % 3]
                                if e is nc.scalar:
                                    e.copy(out=dst, in_=ps[:])
                                else:
                                    e.tensor_copy(out=dst, in_=ps[:])

          # ===== Phase D: attention + proj (hT freed) — with E1 interleaved ===
          with tc.tile_pool(name="pE1p", bufs=1) as pE1p:
            zts = [pE1p.tile([128, DIM], BF, name=f"zts{t}") for t in range(32)]
            hns = [pE1p.tile([128, DIM], BF, name=f"hns{t}") for t in range(32)]

            with tc.tile_pool(name="pD", bufs=6) as pD, \
                 tc.tile_pool(name="pDet", bufs=16) as pDet, \
                 tc.tile_pool(name="pDa", bufs=2) as pDa, \
                 tc.tile_pool(name="pE1", bufs=3) as pE1, \
                 tc.tile_pool(name="pDs_ps", bufs=2, space="PSUM") as pDs_ps, \
                 tc.tile_pool(name="pDo_ps", bufs=2, space="PSUM") as pDo_ps, \
                 tc.tile_pool(name="pDm_ps", bufs=2, space="PSUM") as pDm_ps:

                def e1_group(g):
                    xc = pE1.tile([128, 4, DIM], F32, tag="xe", name="xc")
                    nc.sync.dma_start(
                        out=xc[:],
                        in_=_dram_ap(x_in, 512 * g * DIM,
                                     [[DIM, 128], [128 * DIM, 4], [1, DIM]]))
                    mvall = pE1.tile([128, 4, 2], F32, tag="mva_e", name="mvall")
                    for tt in range(4):
                        yc = pE1.tile([128, DIM], BF, tag="ye", name="yc")
                        nc.sync.dma_start(
                            out=yc[:],
                            in_=_dram_ap(y_d, (8 * g + 2 * tt) * HP * DIM,
                                         [[HP * DIM, 2], [DIM, 64], [1, DIM]]))
                        zt = zts[4 * g + tt]
                        e = [nc.gpsimd, nc.vector][tt % 2]
                        e.tensor_tensor(out=zt[:], in0=xc[:, tt, :], in1=yc[:],
                                        op=ALU.add)
                        stats = pE1.tile([128, 6], F32, tag="st_e", name="stats")
                        nc.vector.bn_stats(out=stats[:], in_=zt[:])
                        nc.vector.bn_aggr(out=mvall[:, tt, :], in_=stats[:])
                    # rstd = rsqrt(var + eps) via Quake + 2 Newton iters (DVE only)
                    vpe = pE1.tile([128, 4], F32, tag="vpe", name="vpe")
                    nc.vector.tensor_scalar(out=vpe[:], in0=_ap(mvall, 1, [[2, 4]]),
                                            scalar1=EPS, scalar2=None, op0=ALU.add)
                    yq = pE1.tile([128, 4], F32, tag="yq", name="yq")
                    sh = yq[:].bitcast(I32)
                    nc.vector.tensor_scalar(out=sh, in0=vpe[:].bitcast(I32),
                                            scalar1=1, scalar2=None,
                                            op0=ALU.arith_shift_right)
                    nc.vector.tensor_tensor(out=sh, in0=magic_t[:].bitcast(I32),
                                            in1=sh, op=ALU.subtract)
                    tq = pE1.tile([128, 4], F32, tag="tq", name="tq")
                    for _ in range(2):
                        nc.vector.tensor_tensor(out=tq[:], in0=vpe[:], in1=yq[:],
                                                op=ALU.mult)
                        nc.vector.tensor_tensor(out=tq[:], in0=tq[:], in1=yq[:],
                                                op=ALU.mult)
                        nc.vector.scalar_tensor_tensor(out=tq[:], in0=tq[:],
                                                       scalar=-0.5, in1=c15_t[:],
                                                       op0=ALU.mult, op1=ALU.add)
                        nc.vector.tensor_tensor(out=yq[:], in0=yq[:], in1=tq[:],
                                                op=ALU.mult)
                    for tt in range(4):
                        nmr = pE1.tile([128, 1], F32, tag="nmr_e", name="nmr")
                        nc.vector.scalar_tensor_tensor(out=nmr[:], in0=mvall[:, tt, 0:1],
                                                       scalar=-1.0, in1=yq[:, tt:tt + 1],
                                                       op0=ALU.mult, op1=ALU.mult)
                        # bf16-in bf16-out SBUF-only: DVE runs this at 4x
                        nc.vector.tensor_scalar(out=hns[4 * g + tt][:],
                                                in0=zts[4 * g + tt][:],
                                                scalar1=nmr[:], scalar2=yq[:, tt:tt + 1],
                                                op0=ALU.add, op1=ALU.mult)

                drr = [0]
                for gi, (p0, plen) in enumerate(GROUPS):
                    nwin = plen // 196
                    kTa = kta2[gi % 2]
                    nc.sync.dma_start(
                        out=kTa[0:64, :, 0:plen],
                        in_=_dram_ap(kT_d, p0,
                                     [[NH * NTOK, 64], [NTOK, NH], [1, plen]]))
                    attnT = pDa.tile([128, 4, 392], F8, tag="attnT", name="attnT")
                    if gi < 2:
                        nc.gpsimd.memset(_ap(attnT, 3 * 392, [[1, 392]], p=(0, 1)), 1.0)
                        nc.gpsimd.memset(_ap(attnT, 3 * 392, [[1, 392]], p=(1, 128)), 0.0)
                    # software-pipelined stages over 3 batches of 2 heads
                    oTs, rzs = {}, {}

                    def stage_a(b):
                        for h in (2 * b, 2 * b + 1):
                            st = pDs_ps.tile([98, 2, 2, 196], F32, tag="st")
                            for i in range(nwin):
                                for j in range(2):
                                    nc.tensor.matmul(
                                        st[:, i, j, :],
                                        kTa[:, h, 196 * i + 98 * j:196 * i + 98 * j + 98],
                                        qb[h][:, p0 + 196 * i:p0 + 196 * i + 196],
                                        start=True, stop=True)
                            et = pDet.tile([98, 2, 2, 196], F8, tag="et")
                            if nwin == 2:
                                nc.scalar.activation(out=et[:], in_=st[:], func=AF.Exp,
                                                     bias=0.0, scale=1.0)
                            else:
                                nc.scalar.activation(out=et[:, 0, :, :],
                                                     in_=st[:, 0, :, :], func=AF.Exp,
                                                     bias=0.0, scale=1.0)
                            ets[h] = et

                    def stage_b(b):
                        for h in (2 * b, 2 * b + 1):
                            oT = pDo_ps.tile([VS, 2, 196], F32, tag="oT")
                            for i in range(nwin):
                                nc.tensor.matmul(
                                    oT[:, i, :],
                                    _ap(vw[2 * gi + i], h * VS,
                                        [[NH * VS, 2], [1, VS]], p=(0, 98)),
                                    ets[h][:, i, :, :], start=True, stop=True,
                                    perf_mode=DR)
                            oTs[h] = oT

                    def stage_c(b):
                        for h in (2 * b, 2 * b + 1):
                            rz = pD.tile([1, 392], BF, tag="rz")
                            i = drr[0]; drr[0] += 1
                            if i % 2 == 0:
                                with nc.allow_low_precision(reason="1/z bf16 ok"):
                                    nc.vector.reciprocal(out=rz[:, 0:196 * nwin],
                                                         in_=oTs[h][64:65, 0:nwin, :])
                            else:
                                nc.gpsimd.tensor_tensor(out=rz[:, 0:196 * nwin],
                                                        in0=onesrow_f[:, 0:196 * nwin],
                                                        in1=oTs[h][64:65, 0:nwin, :],
                                                        op=ALU.divide)
                            rzs[h] = rz

                    def stage_d(b):
                        for h in (2 * b, 2 * b + 1):
                            zt = pDm_ps.tile([98, 512], F32, tag="m")
                            nc.tensor.matmul(zt[0:64, 0:196 * nwin], cb64[:],
                                             rzs[h][:, 0:196 * nwin],
                                             start=True, stop=True)
                            oT = oTs[h]
                            i = drr[0]; drr[0] += 1
                            e = [nc.vector, nc.vector, nc.gpsimd][i % 3]
                            if nwin == 1:
                                dst = _ap(attnT, (h // 2) * 392, [[1, 196]],
                                          p=((h % 2) * 64, (h % 2) * 64 + 64))
                                e.tensor_tensor(out=dst, in0=oT[0:64, 0, :],
                                                in1=_ap(zt, 0, [[1, 196]], p=(0, 64)),
                                                op=ALU.mult)
                            else:
                                dst = _ap(attnT, (h // 2) * 392, [[196, 2], [1, 196]],
                                          p=((h % 2) * 64, (h % 2) * 64 + 64))
                                e.tensor_tensor(out=dst, in0=oT[0:64, :, :],
                                                in1=_ap(zt, 0, [[196, 2], [1, 196]],
                                                        p=(0, 64)),
                                                op=ALU.mult)

                    ets = {}
                    stage_a(0); stage_b(0); stage_c(0)
                    stage_a(1); stage_d(0); stage_b(1); stage_c(1)
                    stage_a(2); stage_d(1); stage_b(2); stage_c(2); stage_d(2)
                    for i in range(nwin):
                        w = 2 * gi + i
                        ysb = pD.tile([98, 2, DIM], BF, tag="ysb")
                        for jj in range(2):
                            pjt = pDm_ps.tile([98, 512], F32, tag="m")
                            pj = pjt[:, 0:DIM]
                            sl = 196 * i + 98 * jj
                            for j in range(2):
                                nc.tensor.matmul(pj,
                                                 attnT[:, 2 * j:2 * j + 2, sl:sl + 98],
                                                 wp_t[:, 2 * j:2 * j + 2, :],
                                                 start=(j == 0), stop=(j == 1),
                                                 perf_mode=DR)
                            k = drr[0]; drr[0] += 1
                            if k % 2 == 0:
                                nc.vector.tensor_scalar(out=ysb[:, jj, :], in0=pj,
                                                        scalar1=FSI, scalar2=None,
                                                        op0=ALU.mult)
                            else:
                                nc.gpsimd.tensor_scalar(out=ysb[:, jj, :], in0=pj,
                                                        scalar1=FSI, scalar2=None,
                                                        op0=ALU.mult)
                        wo = (w // 5) * 14 * HP + (w % 5) * 14
                        for jj in range(2):
                            e = [nc.scalar, nc.sync][jj]
                            e.dma_start(
                                out=_dram_ap(y_d, (wo + 7 * jj * HP) * DIM,
                                             [[HP * DIM, 7], [DIM, 14], [1, DIM]]),
                                in_=ysb[:, jj, :])
                    for g in E1_AFTER.get(gi, []):
                        e1_group(g)

            # ===== Phase E2: fc1 + gelu + fc2 (attention operands freed) =====
            with tc.tile_pool(name="pE2", bufs=3) as pE2, \
                 tc.tile_pool(name="pE2g", bufs=2) as pE2g, \
                 tc.tile_pool(name="pE2h", bufs=2) as pE2h, \
                 tc.tile_pool(name="pE2t_ps", bufs=2, space="PSUM") as pE2t_ps, \
                 tc.tile_pool(name="pE2_ps", bufs=2, space="PSUM") as pE2_ps, \
                 tc.tile_pool(name="pE3_ps", bufs=2, space="PSUM") as pE3_ps:
                for g in range(8):
                    h2T = pE2h.tile([128, 4, 512], F8, tag="h2T", name="h2T")
                    if g < 2:
                        e = [nc.vector, nc.gpsimd][g % 2]
                        e.memset(_ap(h2T, 3 * 512, [[1, 512]], p=(0, 1)), 1.0)
                        e.memset(_ap(h2T, 3 * 512, [[1, 512]], p=(1, 128)), 0.0)
                    for tt in range(4):
                        pt = pE2t_ps.tile([128, 3, 128], BF, tag="htr", name="pt")
                        hn = hns[4 * g + tt]
                        for c in range(3):
                            nc.tensor.transpose(pt[:, c, :], hn[:, c * 128:(c + 1) * 128],
                                                identB[:])
                        dst = _ap(h2T, tt * 128, [[512, 3], [1, 128]])
                        e = [nc.vector, nc.gpsimd][tt % 2]
                        e.tensor_copy(out=dst, in_=pt[:])
                    gt = [pE2g.tile([128, 2, 512], F8, tag=f"g{p}", name=f"g{p}")
                          for p in range(6)]
                    for p in range(6):
                        ps = pE2_ps.tile([128, 2, 512], F32, tag="fc1", name="ps1")
                        for mh in range(2):
                            m = 2 * p + mh
                            for j in range(2):
                                nc.tensor.matmul(
                                    ps[:, mh, :],
                                    w1_t[:, 2 * j:2 * j + 2, m * 128:(m + 1) * 128],
                                    h2T[:, 2 * j:2 * j + 2, :],
                                    start=(j == 0), stop=(j == 1), perf_mode=DR)
                        nc.scalar.activation(out=gt[p][:], in_=ps[:],
                                             func=AF.Gelu, bias=0.0, scale=FSI)
                    ot = pE2.tile([128, 4, DIM], F32, tag="oe", name="ot")
                    for tt in range(4):
                        ps = pE3_ps.tile([128, DIM], F32, tag="fc2", name="ps2")
                        for p in range(6):
                            nc.tensor.matmul(ps[:], gt[p][:, :, tt * 128:(tt + 1) * 128],
                                             w2_t[:, 2 * p:2 * p + 2, :],
                                             start=(p == 0), stop=False, perf_mode=DR)
                        nc.tensor.matmul(ps[:], ones_f8[:], b2row[:],
                                         start=False, stop=True)
                        e = [nc.vector, nc.gpsimd][tt % 2]
                        e.scalar_tensor_tensor(out=ot[:, tt, :], in0=ps[:],
                                               scalar=FSI, in1=zts[4 * g + tt][:],
                                               op0=ALU.mult, op1=ALU.add)
                    nc.sync.dma_start(
                        out=_dram_ap(out_d, 512 * g * DIM,
                                     [[DIM, 128], [128 * DIM, 4], [1, DIM]]),
                        in_=ot[:])

    nc.compile()
    return nc


_NC = None


def _get_nc():
    global _NC
    if _NC is None:
        _NC = build_bass()
    return _NC


def _f8(a):
    return np.ascontiguousarray(
        np.clip(np.asarray(a, np.float32), -240.0, 240.0)).astype(
            ml_dtypes.float8_e4m3)


def _host_prep(inputs):
    f = np.float32
    bf = ml_dtypes.bfloat16
    ln1_w = np.asarray(inputs["ln1_w"], f); ln1_b = np.asarray(inputs["ln1_b"], f)
    qkv_w = np.asarray(inputs["qkv_w"], f); qkv_b = np.asarray(inputs["qkv_b"], f)
    proj_w = np.asarray(inputs["proj_w"], f); proj_b = np.asarray(inputs["proj_b"], f)
    ln2_w = np.asarray(inputs["ln2_w"], f); ln2_b = np.asarray(inputs["ln2_b"], f)
    fc1_w = np.asarray(inputs["fc1_w"], f); fc1_b = np.asarray(inputs["fc1_b"], f)
    fc2_w = np.asarray(inputs["fc2_w"], f); fc2_b = np.asarray(inputs["fc2_b"], f)
    rel_h = np.asarray(inputs["rel_pos_h"], f); rel_w = np.asarray(inputs["rel_pos_w"], f)

    wqk = (ln1_w[:, None] * qkv_w[:, :768]).copy()
    bqk = (ln1_b @ qkv_w[:, :768] + qkv_b[:768]).copy()
    wqk[:, :384] *= SCALE
    bqk[:384] *= SCALE
    wv = (ln1_w[:, None] * qkv_w[:, 768:]).copy()
    bv = ln1_b @ qkv_w[:, 768:] + qkv_b[768:]

    def chunk4(wmat, n, bias_row):
        # [384, n] -> [128, 4, n]: chunks 0..2 = w rows, chunk3 row0 = bias
        out = np.zeros((128, 4, n), f)
        for kc in range(3):
            out[:, kc, :] = wmat[kc * 128:(kc + 1) * 128, :]
        out[0, 3, :] = bias_row
        return out * FS

    wqk4 = chunk4(wqk, 768, np.concatenate([bqk[:384], np.zeros(384, f)]))
    wv4 = chunk4(wv, 384, np.zeros(384, f))
    bp = proj_b + bv @ proj_w
    wp4 = chunk4(proj_w, 384, bp)
    w1m = ln2_w[:, None] * fc1_w
    b1 = ln2_b @ fc1_w + fc1_b
    w14 = chunk4(w1m, MLP, b1)
    w2m = np.zeros((128, 12, DIM), f)
    for kc in range(12):
        w2m[:, kc, :] = fc2_w[kc * 128:(kc + 1) * 128, :]
    w2m *= FS

    coords = np.arange(WS)[:, None] - np.arange(WS)[None, :] + (WS - 1)
    Rh = rel_h[coords]
    Rw = rel_w[coords]
    rel = np.zeros((HD, 2 * 196), f)
    for r in range(14):
        rel[:, r * 14:(r + 1) * 14] = Rh[r].T / SCALE
    for c in range(14):
        rel[:, 196 + c * 14:196 + (c + 1) * 14] = Rw[c].T / SCALE

    kpat = np.zeros((28, 392), f)
    for j in range(14):
        for a in range(2):
            kpat[j, 196 * a + 14 * j:196 * a + 14 * j + 14] = 1.0
            kpat[14 + j, 196 * a + j::14][:14] = 1.0

    return {
        "wqk": _f8(wqk4.reshape(128, -1)),
        "wv": _f8(wv4.reshape(128, -1)),
        "rel": rel.astype(bf),
        "kpat": kpat.astype(bf),
        "wp": _f8(wp4.reshape(128, -1)),
        "w1": _f8(w14.reshape(128, -1)),
        "w2": _f8(w2m.reshape(128, -1)),
        "b2": _f8(fc2_b * FS),
    }


def kernel(**inputs):
    nc = _get_nc()
    shared = _host_prep(inputs)
    x = np.asarray(inputs["x"], np.float32).reshape(B, NVAL, DIM)
    in_maps = [dict(shared, x=np.ascontiguousarray(x[c])) for c in range(B)]
    res = run_bass_kernel_spmd(nc, in_maps, list(range(B)))
    out = np.stack([res.results[c]["out"] for c in range(B)])
    return out.reshape(B, H, W, DIM)


if __name__ == "__main__":
    build_bass()
    print("build ok")


# revision 28
# speedup vs baseline: 1.2627x; 1.0400x over previous
"""Bass/Trainium2 kernel v3 for nn_BlockForNormalWindow (windowed-attention
transformer block), data-parallel over batch across 8 NeuronCores.

v3 over v2: fp8e4 DoubleRow matmuls for qkv/v/proj/fc1/fc2 (weights x64,
biases folded via ones-row in a 4th K-chunk), window-major fp8 hT
(contiguous group slices, no hstage), bf16 score path with K=92 layout
(no garbage rows), fp8 ets/v with DoubleRow AV, PE-broadcast of 1/z,
Quake rsqrt on DVE for LN2 (no act-table thrash), 3-way engine rotation
for PSUM->SBUF copies, split E1/E2 MLP phase."""
import sys
sys.path.insert(0, '/opt/trn_rl_repo')

import numpy as np
import ml_dtypes
import concourse.bass as bass
import concourse.mybir as mybir
import concourse.tile as tile
from concourse import bacc
from concourse.bass_utils import run_bass_kernel_spmd
from concourse.masks import make_identity

F32 = mybir.dt.float32
I32 = mybir.dt.int32
BF = mybir.dt.bfloat16
F8 = mybir.dt.float8e4
AF = mybir.ActivationFunctionType
ALU = mybir.AluOpType
DR = mybir.MatmulPerfMode.DoubleRow

B, H, W = 8, 64, 64
DIM, NH, WS = 384, 6, 14
HD = DIM // NH
MLP = 4 * DIM
EPS = 1e-5
SCALE = HD ** -0.5
HP = 70
NWIN = 25
NTOK = NWIN * WS * WS        # 4900
NVAL = H * W                 # 4096
VS = 65                      # per-head stride in v layout (64 vals + ones col)
KR = 92                      # rows in k/q operand: q/k 0:64, relh 64:78, relw 78:92
FS = 64.0                    # fp8 weight pre-scale
FSI = 1.0 / FS
QMAGIC = 1.3211836172961055e+19   # 0x5f3759df as float32

GROUPS = [(g * 392, 392) for g in range(12)] + [(4704, 196)]
# E1 group g (image rows 8g:8g+8) ready after this D group index
E1_AFTER = {2: [0], 4: [1, 2], 7: [3, 4], 9: [5, 6], 12: [7]}


def _ap(t, offset_elems, dims, p=None):
    a = t[:, 0:1] if p is None else t[p[0]:p[1], 0:1]
    return bass.AP(tensor=a.tensor, offset=a.offset + offset_elems,
                   ap=[a.ap[0]] + dims)


def _dram_ap(t, offset_elems, dims):
    a = t.ap()
    return bass.AP(tensor=a.tensor, offset=offset_elems, ap=dims)


def build_bass():
    nc = bacc.Bacc("TRN2", target_bir_lowering=False, debug=False)

    x_in = nc.dram_tensor("x", [NVAL, DIM], F32, kind="ExternalInput")
    wqk_in = nc.dram_tensor("wqk", [128, 4 * 2 * DIM], F8, kind="ExternalInput")
    wv_in = nc.dram_tensor("wv", [128, 4 * DIM], F8, kind="ExternalInput")
    rel_in = nc.dram_tensor("rel", [HD, 2 * 196], BF, kind="ExternalInput")
    kpat_in = nc.dram_tensor("kpat", [28, 392], BF, kind="ExternalInput")
    wp_in = nc.dram_tensor("wp", [128, 4 * DIM], F8, kind="ExternalInput")
    w1_in = nc.dram_tensor("w1", [128, 4 * MLP], F8, kind="ExternalInput")
    w2_in = nc.dram_tensor("w2", [128, 12 * DIM], F8, kind="ExternalInput")
    b2_in = nc.dram_tensor("b2", [DIM], F8, kind="ExternalInput")
    out_d = nc.dram_tensor("out", [NVAL, DIM], F32, kind="ExternalOutput")

    # k operand in DRAM: rows 0:64 x [NH, NTOK] bf16, written in phase B.
    kT_d = nc.dram_tensor("kT_d", [64, NH * NTOK], BF)
    y_d = nc.dram_tensor("y_d", [HP * HP, DIM], BF)

    with tile.TileContext(nc) as tc:
      with tc.tile_pool(name="singles", bufs=1) as singles:
        ident_f = singles.tile([128, 128], F32)
        make_identity(nc, ident_f[:])
        identB = singles.tile([128, 128], BF)
        nc.vector.tensor_copy(out=identB[:], in_=ident_f[:])

        eps_t = singles.tile([128, 1], F32)
        nc.vector.memset(eps_t[:], EPS)
        cb64 = singles.tile([1, 64], BF)
        nc.gpsimd.memset(cb64[:], 1.0)
        ones_f8 = singles.tile([1, 128], F8)
        nc.gpsimd.memset(ones_f8[:], 1.0)
        onesrow_f = singles.tile([1, 392], F32)
        nc.gpsimd.memset(onesrow_f[:], 1.0)
        magic_t = singles.tile([128, 4], F32)
        nc.vector.memset(magic_t[:], QMAGIC)
        c15_t = singles.tile([128, 4], F32)
        nc.vector.memset(c15_t[:], 1.5)

        # weights
        wqk_t = singles.tile([128, 4, 2 * DIM], F8)
        nc.sync.dma_start(out=wqk_t[:], in_=wqk_in.ap())
        wv_t = singles.tile([128, 4, DIM], F8)
        nc.sync.dma_start(out=wv_t[:], in_=wv_in.ap())
        relm_t = singles.tile([HD, 2 * 196], BF)
        nc.sync.dma_start(out=relm_t[:], in_=rel_in.ap())
        wp_t = singles.tile([128, 4, DIM], F8)
        nc.sync.dma_start(out=wp_t[:], in_=wp_in.ap())
        w1_t = singles.tile([128, 4, MLP], F8)
        nc.sync.dma_start(out=w1_t[:], in_=w1_in.ap())
        w2_t = singles.tile([128, 12, DIM], F8)
        nc.sync.dma_start(out=w2_t[:], in_=w2_in.ap())
        b2row = singles.tile([1, DIM], F8)
        nc.sync.dma_start(out=b2row[:], in_=b2_in.ap())

        with tc.tile_pool(name="attops", bufs=1) as attops:
          qb = [attops.tile([KR, NTOK], BF, name=f"qb{h}") for h in range(NH)]
          vw = [attops.tile([98, 2, NH * VS], F8, name=f"vw{w}") for w in range(NWIN)]
          for w in range(NWIN):
              e = [nc.vector, nc.gpsimd][w % 2]
              e.memset(_ap(vw[w], 64, [[NH * VS, 2], [VS, NH], [1, 1]]), 1.0)
          kta2 = [attops.tile([KR, NH, 392], BF, name=f"kta{i}") for i in range(2)]
          for i in range(2):
              nc.sync.dma_start(
                  out=kta2[i][64:KR, :, :],
                  in_=bass.AP(tensor=kpat_in.ap().tensor, offset=0,
                              ap=[[392, 28], [0, NH], [1, 392]]))

          with tc.tile_pool(name="pHT", bufs=1) as pHT:
            hT = pHT.tile([128, 4, NTOK], F8, name="hT")
            # chunk 3: row0 = 1.0 (bias row), rows 1:128 = 0; 3-way col split
            for i in range(4):
                e = [nc.vector, nc.gpsimd][i % 2]
                c0 = i * 1225
                e.memset(_ap(hT, 3 * NTOK + c0, [[1, 1225]], p=(0, 1)), 1.0)
                e.memset(_ap(hT, 3 * NTOK + c0, [[1, 1225]], p=(1, 128)), 0.0)
            # zero padding tokens in chunks 0:3 (right-edge and bottom windows)
            for c in range(3):
                e = [nc.vector, nc.gpsimd][c % 2]
                # right-edge windows w%5==4, cols 8:14 of each window row
                e.memset(_ap(hT, c * NTOK + 4 * 196 + 8, [[980, 5], [14, 14], [1, 6]]),
                         0.0)
                # bottom windows 20..24, rows 8:14
                e.memset(_ap(hT, c * NTOK + 20 * 196 + 8 * 14, [[196, 5], [1, 84]]),
                         0.0)

            # ===== Phase A: LN1 + transpose into window-major fp8 hT =====
            with tc.tile_pool(name="pA", bufs=4) as pA, \
                 tc.tile_pool(name="pA_ps", bufs=4, space="PSUM") as pA_ps:
                for ch in range(8):
                    xc = pA.tile([128, 4, DIM], F32, tag="xc")
                    nc.sync.dma_start(
                        out=xc[:],
                        in_=_dram_ap(x_in, 512 * ch * DIM,
                                     [[DIM, 128], [128 * DIM, 4], [1, DIM]]))
                    mvall = pA.tile([128, 4, 2], F32, tag="mva")
                    for tt in range(4):
                        stats = pA.tile([128, 6], F32, tag="st")
                        nc.vector.bn_stats(out=stats[:], in_=xc[:, tt, :])
                        nc.vector.bn_aggr(out=mvall[:, tt, :], in_=stats[:])
                    rstd = pA.tile([128, 4], F32, tag="rstd")
                    nc.scalar.activation(out=rstd[:], in_=_ap(mvall, 1, [[2, 4]]),
                                         func=AF.Sqrt, bias=eps_t[:], scale=1.0)
                    nc.vector.reciprocal(out=rstd[:], in_=rstd[:])
                    for tt in range(4):
                        t = 4 * ch + tt
                        nmr = pA.tile([128, 1], F32, tag="nmr")
                        nc.vector.scalar_tensor_tensor(out=nmr[:], in0=mvall[:, tt, 0:1],
                                                       scalar=-1.0, in1=rstd[:, tt:tt + 1],
                                                       op0=ALU.mult, op1=ALU.mult)
                        hn = pA.tile([128, DIM], BF, tag="hn")
                        nc.scalar.activation(out=hn[:], in_=xc[:, tt, :],
                                             func=AF.Identity, bias=nmr[:],
                                             scale=rstd[:, tt:tt + 1])
                        pt = pA_ps.tile([128, 3, 128], BF, tag="tr")
                        for c in range(3):
                            nc.tensor.transpose(pt[:, c, :], hn[:, c * 128:(c + 1) * 128],
                                                identB[:])
                        # window-major scatter: rows r0, r0+1 of the image
                        # (4 full col-windows of 14 + 1 partial of 8)
                        r0 = 2 * t
                        band, rb = r0 // 14, r0 % 14
                        e = [nc.gpsimd, nc.vector][t % 2]
                        dst = _ap(hT, band * 980 + rb * 14,
                                  [[NTOK, 3], [14, 2], [196, 4], [1, 14]])
                        src = bass.AP(tensor=pt[:].tensor, offset=pt[:].offset,
                                      ap=[pt[:].ap[0], [128, 3], [64, 2], [1, 56]])
                        if t % 2 == 0:
                            nc.scalar.copy(out=dst, in_=src)
                        else:
                            nc.gpsimd.tensor_copy(out=dst, in_=src)
                        dst = _ap(hT, band * 980 + 4 * 196 + rb * 14,
                                  [[NTOK, 3], [14, 2], [1, 8]])
                        src = bass.AP(tensor=pt[:].tensor, offset=pt[:].offset + 56,
                                      ap=[pt[:].ap[0], [128, 3], [64, 2], [1, 8]])
                        e.tensor_copy(out=dst, in_=src)

            # ===== Phase B: qk DR matmuls, rel, v =====
            with tc.tile_pool(name="pB", bufs=3) as pB, \
                 tc.tile_pool(name="pB_ps", bufs=2, space="PSUM") as pB_ps, \
                 tc.tile_pool(name="pC_ps", bufs=4, space="PSUM") as pC_ps, \
                 tc.tile_pool(name="pBv_ps", bufs=2, space="PSUM") as pBv_ps:
                copy_rr = [0]

                def scaled_copy(dst, src):
                    i = copy_rr[0]; copy_rr[0] += 1
                    if i % 3 == 0:
                        nc.scalar.activation(out=dst, in_=src, func=AF.Identity,
                                             bias=0.0, scale=FSI)
                    elif i % 3 == 1:
                        nc.vector.tensor_scalar(out=dst, in0=src, scalar1=FSI,
                                                scalar2=None, op0=ALU.mult)
                    else:
                        nc.gpsimd.tensor_scalar(out=dst, in0=src, scalar1=FSI,
                                                scalar2=None, op0=ALU.mult)

                def emit_v(s_):
                    w, half = s_ // 2, s_ % 2
                    ps = pBv_ps.tile([98, DIM], F32, tag="v", name="vps")
                    for j in range(2):
                        nc.tensor.matmul(ps[:],
                                         _ap(hT, 2 * j * NTOK + 98 * s_,
                                             [[NTOK, 2], [1, 98]]),
                                         wv_t[:, 2 * j:2 * j + 2, :],
                                         start=(j == 0), stop=(j == 1), perf_mode=DR)
                    scaled_copy(_ap(vw[w], half * NH * VS, [[VS, NH], [1, 64]]), ps[:])

                vnext = [0]
                relc = [0]
                pending_rel = []

                def rel_op(h, r, is_row):
                    def go():
                        ps = pC_ps.tile([14, 350], F32, tag="rel")
                        if is_row:
                            nc.tensor.matmul(
                                ps[:], relm_t[:, r * 14:(r + 1) * 14],
                                _ap(qb[h], r * 14, [[196, 25], [1, 14]], p=(0, 64)),
                                start=True, stop=True)
                            dst = _ap(qb[h], r * 14, [[196, 25], [1, 14]], p=(64, 78))
                        else:
                            nc.tensor.matmul(
                                ps[:], relm_t[:, 196 + r * 14:196 + (r + 1) * 14],
                                _ap(qb[h], r, [[196, 25], [14, 14]], p=(0, 64)),
                                start=True, stop=True)
                            dst = _ap(qb[h], r, [[196, 25], [14, 14]], p=(78, 92))
                        i = relc[0]; relc[0] += 1
                        e = [nc.scalar, nc.vector, nc.gpsimd][i % 3]
                        if e is nc.scalar:
                            e.copy(out=dst, in_=ps[:])
                        else:
                            e.tensor_copy(out=dst, in_=ps[:])
                    return go

                for m in range(6):
                    for gi, (p0, plen) in enumerate(GROUPS):
                        ps = pB_ps.tile([128, 392], F32, tag="qk")
                        for j in range(2):
                            nc.tensor.matmul(
                                ps[:, 0:plen],
                                wqk_t[:, 2 * j:2 * j + 2, m * 128:(m + 1) * 128],
                                _ap(hT, 2 * j * NTOK + p0, [[NTOK, 2], [1, plen]]),
                                start=(j == 0), stop=(j == 1), perf_mode=DR)
                        if m < 3:
                            for half in range(2):
                                h = 2 * m + half
                                scaled_copy(qb[h][0:64, p0:p0 + plen],
                                            ps[64 * half:64 * half + 64, 0:plen])
                        else:
                            mm = m - 3
                            kst = pB.tile([128, 392], BF, tag="kst", bufs=4,
                                          name="kst")
                            scaled_copy(kst[:, 0:plen], ps[:, 0:plen])
                            nc.sync.dma_start(
                                out=_dram_ap(kT_d, 2 * mm * NTOK + p0,
                                             [[NTOK, 2], [NH * NTOK, 64], [1, plen]]),
                                in_=kst[:, 0:plen])
                        if vnext[0] < 50 and (m, gi) != (0, 0):
                            emit_v(vnext[0])
                            vnext[0] += 1
                        for _ in range(5):
                            if pending_rel:
                                pending_rel.pop(0)()
                    if m < 3:
                        # queue rel rows for heads 2m, 2m+1 (q complete now);
                        # they interleave into the next m's group loop
                        for r in range(14):
                            for half in range(2):
                                h = 2 * m + half
                                pending_rel.append(rel_op(h, r, True))
                                pending_rel.append(rel_op(h, r, False))
                while pending_rel:
                    pending_rel.pop(0)()

          # ===== Phase D: attention + proj (hT freed) — with E1 interleaved ===
          with tc.tile_pool(name="pE1p", bufs=1) as pE1p:
            zts = [pE1p.tile([128, DIM], BF, name=f"zts{t}") for t in range(32)]
            hns = [pE1p.tile([128, DIM], BF, name=f"hns{t}") for t in range(32)]

            with tc.tile_pool(name="pD", bufs=6) as pD, \
                 tc.tile_pool(name="pDet", bufs=16) as pDet, \
                 tc.tile_pool(name="pDa", bufs=2) as pDa, \
                 tc.tile_pool(name="pE1", bufs=3) as pE1, \
                 tc.tile_pool(name="pDs_ps", bufs=2, space="PSUM") as pDs_ps, \
                 tc.tile_pool(name="pDo_ps", bufs=2, space="PSUM") as pDo_ps, \
                 tc.tile_pool(name="pDm_ps", bufs=2, space="PSUM") as pDm_ps:

                def e1_group(g):
                    xc = pE1.tile([128, 4, DIM], F32, tag="xe", name="xc")
                    nc.sync.dma_start(
                        out=xc[:],
                        in_=_dram_ap(x_in, 512 * g * DIM,
                                     [[DIM, 128], [128 * DIM, 4], [1, DIM]]))
                    mvall = pE1.tile([128, 4, 2], F32, tag="mva_e", name="mvall")
                    for tt in range(4):
                        yc = pE1.tile([128, DIM], BF, tag="ye", name="yc")
                        nc.sync.dma_start(
                            out=yc[:],
                            in_=_dram_ap(y_d, (8 * g + 2 * tt) * HP * DIM,
                                         [[HP * DIM, 2], [DIM, 64], [1, DIM]]))
                        zt = zts[4 * g + tt]
                        e = [nc.gpsimd, nc.vector][tt % 2]
                        e.tensor_tensor(out=zt[:], in0=xc[:, tt, :], in1=yc[:],
                                        op=ALU.add)
                        stats = pE1.tile([128, 6], F32, tag="st_e", name="stats")
                        nc.vector.bn_stats(out=stats[:], in_=zt[:])
                        nc.vector.bn_aggr(out=mvall[:, tt, :], in_=stats[:])
                    # rstd = rsqrt(var + eps) via Quake + 2 Newton iters (DVE only)
                    vpe = pE1.tile([128, 4], F32, tag="vpe", name="vpe")
                    nc.vector.tensor_scalar(out=vpe[:], in0=_ap(mvall, 1, [[2, 4]]),
                                            scalar1=EPS, scalar2=None, op0=ALU.add)
                    yq = pE1.tile([128, 4], F32, tag="yq", name="yq")
                    sh = yq[:].bitcast(I32)
                    nc.vector.tensor_scalar(out=sh, in0=vpe[:].bitcast(I32),
                                            scalar1=1, scalar2=None,
                                            op0=ALU.arith_shift_right)
                    nc.vector.tensor_tensor(out=sh, in0=magic_t[:].bitcast(I32),
                                            in1=sh, op=ALU.subtract)
                    tq = pE1.tile([128, 4], F32, tag="tq", name="tq")
                    for _ in range(2):
                        nc.vector.tensor_tensor(out=tq[:], in0=vpe[:], in1=yq[:],
                                                op=ALU.mult)
                        nc.vector.tensor_tensor(out=tq[:], in0=tq[:], in1=yq[:],
                                                op=ALU.mult)
                        nc.vector.scalar_tensor_tensor(out=tq[:], in0=tq[:],
                                                       scalar=-0.5, in1=c15_t[:],
                                                       op0=ALU.mult, op1=ALU.add)
                        nc.vector.tensor_tensor(out=yq[:], in0=yq[:], in1=tq[:],
                                                op=ALU.mult)
                    for tt in range(4):
                        nmr = pE1.tile([128, 1], F32, tag="nmr_e", name="nmr")
                        nc.vector.scalar_tensor_tensor(out=nmr[:], in0=mvall[:, tt, 0:1],
                                                       scalar=-1.0, in1=yq[:, tt:tt + 1],
                                                       op0=ALU.mult, op1=ALU.mult)
                        # bf16-in bf16-out SBUF-only: DVE runs this at 4x
                        nc.vector.tensor_scalar(out=hns[4 * g + tt][:],
                                                in0=zts[4 * g + tt][:],
                                                scalar1=nmr[:], scalar2=yq[:, tt:tt + 1],
                                                op0=ALU.add, op1=ALU.mult)

                drr = [0]
                for gi, (p0, plen) in enumerate(GROUPS):
                    nwin = plen // 196
                    kTa = kta2[gi % 2]
                    nc.sync.dma_start(
                        out=kTa[0:64, :, 0:plen],
                        in_=_dram_ap(kT_d, p0,
                                     [[NH * NTOK, 64], [NTOK, NH], [1, plen]]))
                    attnT = pDa.tile([128, 4, 392], F8, tag="attnT", name="attnT")
                    if gi < 2:
                        nc.gpsimd.memset(_ap(attnT, 3 * 392, [[1, 392]], p=(0, 1)), 1.0)
                        nc.gpsimd.memset(_ap(attnT, 3 * 392, [[1, 392]], p=(1, 128)), 0.0)
                    # software-pipelined stages over 3 batches of 2 heads
                    oTs, rzs = {}, {}

                    def stage_a(b):
                        for h in (2 * b, 2 * b + 1):
                            st = pDs_ps.tile([98, 2, 2, 196], F32, tag="st")
                            for i in range(nwin):
                                for j in range(2):
                                    nc.tensor.matmul(
                                        st[:, i, j, :],
                                        kTa[:, h, 196 * i + 98 * j:196 * i + 98 * j + 98],
                                        qb[h][:, p0 + 196 * i:p0 + 196 * i + 196],
                                        start=True, stop=True)
                            et = pDet.tile([98, 2, 2, 196], F8, tag="et")
                            if nwin == 2:
                                nc.scalar.activation(out=et[:], in_=st[:], func=AF.Exp,
                                                     bias=0.0, scale=1.0)
                            else:
                                nc.scalar.activation(out=et[:, 0, :, :],
                                                     in_=st[:, 0, :, :], func=AF.Exp,
                                                     bias=0.0, scale=1.0)
                            ets[h] = et

                    def stage_b(b):
                        for h in (2 * b, 2 * b + 1):
                            oT = pDo_ps.tile([VS, 2, 196], F32, tag="oT")
                            for i in range(nwin):
                                nc.tensor.matmul(
                                    oT[:, i, :],
                                    _ap(vw[2 * gi + i], h * VS,
                                        [[NH * VS, 2], [1, VS]], p=(0, 98)),
                                    ets[h][:, i, :, :], start=True, stop=True,
                                    perf_mode=DR)
                            oTs[h] = oT

                    def stage_c(b):
                        for h in (2 * b, 2 * b + 1):
                            rz = pD.tile([1, 392], BF, tag="rz")
                            i = drr[0]; drr[0] += 1
                            if i % 2 == 0:
                                with nc.allow_low_precision(reason="1/z bf16 ok"):
                                    nc.vector.reciprocal(out=rz[:, 0:196 * nwin],
                                                         in_=oTs[h][64:65, 0:nwin, :])
                            else:
                                nc.gpsimd.tensor_tensor(out=rz[:, 0:196 * nwin],
                                                        in0=onesrow_f[:, 0:196 * nwin],
                                                        in1=oTs[h][64:65, 0:nwin, :],
                                                        op=ALU.divide)
                            rzs[h] = rz

                    def stage_d(b):
                        for h in (2 * b, 2 * b + 1):
                            zt = pDm_ps.tile([98, 512], F32, tag="m")
                            nc.tensor.matmul(zt[0:64, 0:196 * nwin], cb64[:],
                                             rzs[h][:, 0:196 * nwin],
                                             start=True, stop=True)
                            oT = oTs[h]
                            i = drr[0]; drr[0] += 1
                            e = [nc.vector, nc.vector, nc.gpsimd][i % 3]
                            if nwin == 1:
                                dst = _ap(attnT, (h // 2) * 392, [[1, 196]],
                                          p=((h % 2) * 64, (h % 2) * 64 + 64))
                                e.tensor_tensor(out=dst, in0=oT[0:64, 0, :],
                                                in1=_ap(zt, 0, [[1, 196]], p=(0, 64)),
                                                op=ALU.mult)
                            else:
                                dst = _ap(attnT, (h // 2) * 392, [[196, 2], [1, 196]],
                                          p=((h % 2) * 64, (h % 2) * 64 + 64))
                                e.tensor_tensor(out=dst, in0=oT[0:64, :, :],
                                                in1=_ap(zt, 0, [[196, 2], [1, 196]],
                                                        p=(0, 64)),
                                                op=ALU.mult)

                    ets = {}
                    stage_a(0); stage_b(0); stage_c(0)
                    stage_a(1); stage_d(0); stage_b(1); stage_c(1)
                    stage_a(2); stage_d(1); stage_b(2); stage_c(2); stage_d(2)
                    for i in range(nwin):
                        w = 2 * gi + i
                        ysb = pD.tile([98, 2, DIM], BF, tag="ysb")
                        for jj in range(2):
                            pjt = pDm_ps.tile([98, 512], F32, tag="m")
                            pj = pjt[:, 0:DIM]
                            sl = 196 * i + 98 * jj
                            for j in range(2):
                                nc.tensor.matmul(pj,
                                                 attnT[:, 2 * j:2 * j + 2, sl:sl + 98],
                                                 wp_t[:, 2 * j:2 * j + 2, :],
                                                 start=(j == 0), stop=(j == 1),
                                                 perf_mode=DR)
                            nc.scalar.activation(out=ysb[:, jj, :], in_=pj,
                                                 func=AF.Identity, bias=0.0,
                                                 scale=FSI)
                        wo = (w // 5) * 14 * HP + (w % 5) * 14
                        for jj in range(2):
                            e = [nc.scalar, nc.sync][jj]
                            e.dma_start(
                                out=_dram_ap(y_d, (wo + 7 * jj * HP) * DIM,
                                             [[HP * DIM, 7], [DIM, 14], [1, DIM]]),
                                in_=ysb[:, jj, :])
                    for g in E1_AFTER.get(gi, []):
                        e1_group(g)

            # ===== Phase E2: fc1 + gelu + fc2 (attention operands freed) =====
            with tc.tile_pool(name="pE2", bufs=3) as pE2, \
                 tc.tile_pool(name="pE2g", bufs=2) as pE2g, \
                 tc.tile_pool(name="pE2h", bufs=2) as pE2h, \
                 tc.tile_pool(name="pE2t_ps", bufs=2, space="PSUM") as pE2t_ps, \
                 tc.tile_pool(name="pE2_ps", bufs=2, space="PSUM") as pE2_ps, \
                 tc.tile_pool(name="pE3_ps", bufs=2, space="PSUM") as pE3_ps:
                for g in range(8):
                    h2T = pE2h.tile([128, 4, 512], F8, tag="h2T", name="h2T")
                    if g < 2:
                        e = [nc.vector, nc.gpsimd][g % 2]
                        e.memset(_ap(h2T, 3 * 512, [[1, 512]], p=(0, 1)), 1.0)
                        e.memset(_ap(h2T, 3 * 512, [[1, 512]], p=(1, 128)), 0.0)
                    for tt in range(4):
                        pt = pE2t_ps.tile([128, 3, 128], BF, tag="htr", name="pt")
                        hn = hns[4 * g + tt]
                        for c in range(3):
                            nc.tensor.transpose(pt[:, c, :], hn[:, c * 128:(c + 1) * 128],
                                                identB[:])
                        dst = _ap(h2T, tt * 128, [[512, 3], [1, 128]])
                        e = [nc.vector, nc.gpsimd][tt % 2]
                        e.tensor_copy(out=dst, in_=pt[:])
                    gt = [pE2g.tile([128, 2, 512], F8, tag=f"g{p}", name=f"g{p}")
                          for p in range(6)]
                    for p in range(6):
                        ps = pE2_ps.tile([128, 2, 512], F32, tag="fc1", name="ps1")
                        for mh in range(2):
                            m = 2 * p + mh
                            for j in range(2):
                                nc.tensor.matmul(
                                    ps[:, mh, :],
                                    w1_t[:, 2 * j:2 * j + 2, m * 128:(m + 1) * 128],
                                    h2T[:, 2 * j:2 * j + 2, :],
                                    start=(j == 0), stop=(j == 1), perf_mode=DR)
                        nc.scalar.activation(out=gt[p][:], in_=ps[:],
                                             func=AF.Gelu, bias=0.0, scale=FSI)
                    ot = pE2.tile([128, 4, DIM], F32, tag="oe", name="ot")
                    for tt in range(4):
                        ps = pE3_ps.tile([128, DIM], F32, tag="fc2", name="ps2")
                        for p in range(6):
                            nc.tensor.matmul(ps[:], gt[p][:, :, tt * 128:(tt + 1) * 128],
                                             w2_t[:, 2 * p:2 * p + 2, :],
                                             start=(p == 0), stop=False, perf_mode=DR)
                        nc.tensor.matmul(ps[:], ones_f8[:], b2row[:],
                                         start=False, stop=True)
                        e = [nc.vector, nc.gpsimd][tt % 2]
                        e.scalar_tensor_tensor(out=ot[:, tt, :], in0=ps[:],
                                               scalar=FSI, in1=zts[4 * g + tt][:],
                                               op0=ALU.mult, op1=ALU.add)
                    nc.sync.dma_start(
                        out=_dram_ap(out_d, 512 * g * DIM,
                                     [[DIM, 128], [128 * DIM, 4], [1, DIM]]),
                        in_=ot[:])

    nc.compile()
    return nc


_NC = None


def _get_nc():
    global _NC
    if _NC is None:
        _NC = build_bass()
    return _NC


def _f8(a):
    return np.ascontiguousarray(
        np.clip(np.asarray(a, np.float32), -240.0, 240.0)).astype(
            ml_dtypes.float8_e4m3)


def _host_prep(inputs):
    f = np.float32
    bf = ml_dtypes.bfloat16
    ln1_w = np.asarray(inputs["ln1_w"], f); ln1_b = np.asarray(inputs["ln1_b"], f)
    qkv_w = np.asarray(inputs["qkv_w"], f); qkv_b = np.asarray(inputs["qkv_b"], f)
    proj_w = np.asarray(inputs["proj_w"], f); proj_b = np.asarray(inputs["proj_b"], f)
    ln2_w = np.asarray(inputs["ln2_w"], f); ln2_b = np.asarray(inputs["ln2_b"], f)
    fc1_w = np.asarray(inputs["fc1_w"], f); fc1_b = np.asarray(inputs["fc1_b"], f)
    fc2_w = np.asarray(inputs["fc2_w"], f); fc2_b = np.asarray(inputs["fc2_b"], f)
    rel_h = np.asarray(inputs["rel_pos_h"], f); rel_w = np.asarray(inputs["rel_pos_w"], f)

    wqk = (ln1_w[:, None] * qkv_w[:, :768]).copy()
    bqk = (ln1_b @ qkv_w[:, :768] + qkv_b[:768]).copy()
    wqk[:, :384] *= SCALE
    bqk[:384] *= SCALE
    wv = (ln1_w[:, None] * qkv_w[:, 768:]).copy()
    bv = ln1_b @ qkv_w[:, 768:] + qkv_b[768:]

    def chunk4(wmat, n, bias_row):
        # [384, n] -> [128, 4, n]: chunks 0..2 = w rows, chunk3 row0 = bias
        out = np.zeros((128, 4, n), f)
        for kc in range(3):
            out[:, kc, :] = wmat[kc * 128:(kc + 1) * 128, :]
        out[0, 3, :] = bias_row
        return out * FS

    wqk4 = chunk4(wqk, 768, np.concatenate([bqk[:384], np.zeros(384, f)]))
    wv4 = chunk4(wv, 384, np.zeros(384, f))
    bp = proj_b + bv @ proj_w
    wp4 = chunk4(proj_w, 384, bp)
    w1m = ln2_w[:, None] * fc1_w
    b1 = ln2_b @ fc1_w + fc1_b
    w14 = chunk4(w1m, MLP, b1)
    w2m = np.zeros((128, 12, DIM), f)
    for kc in range(12):
        w2m[:, kc, :] = fc2_w[kc * 128:(kc + 1) * 128, :]
    w2m *= FS

    coords = np.arange(WS)[:, None] - np.arange(WS)[None, :] + (WS - 1)
    Rh = rel_h[coords]
    Rw = rel_w[coords]
    rel = np.zeros((HD, 2 * 196), f)
    for r in range(14):
        rel[:, r * 14:(r + 1) * 14] = Rh[r].T / SCALE
    for c in range(14):
        rel[:, 196 + c * 14:196 + (c + 1) * 14] = Rw[c].T / SCALE

    kpat = np.zeros((28, 392), f)
    for j in range(14):
        for a in range(2):
            kpat[j, 196 * a + 14 * j:196 * a + 14 * j + 14] = 1.0
            kpat[14 + j, 196 * a + j::14][:14] = 1.0

    return {
        "wqk": _f8(wqk4.reshape(128, -1)),
        "wv": _f8(wv4.reshape(128, -1)),
        "rel": rel.astype(bf),
        "kpat": kpat.astype(bf),
        "wp": _f8(wp4.reshape(128, -1)),
        "w1": _f8(w14.reshape(128, -1)),
        "w2": _f8(w2m.reshape(128, -1)),
        "b2": _f8(fc2_b * FS),
    }


def kernel(**inputs):
    nc = _get_nc()
    shared = _host_prep(inputs)
    x = np.asarray(inputs["x"], np.float32).reshape(B, NVAL, DIM)
    in_maps = [dict(shared, x=np.ascontiguousarray(x[c])) for c in range(B)]
    res = run_bass_kernel_spmd(nc, in_maps, list(range(B)))
    out = np.stack([res.results[c]["out"] for c in range(B)])
    return out.reshape(B, H, W, DIM)


if __name__ == "__main__":
    build_bass()
    print("build ok")
